# revision 43
# baseline (speedup 1.0000x reference)
"""Trainium2 Bass kernel for nn_AA_encoder (gnn_message_passing).

Data-parallel over the B=64 graph dimension: 8 graphs per NeuronCore on a
single TRN2 chip (8 cores).  Per-core pipeline:

  P1  masked BERT token-sum (PE selector matmuls over host-pre-split
      bf16 hi/lo BERT pairs) -> tok_sum, transposed to tok_sumT pairs
  P1b dense projection (3-term bf16 split product, batched over graphs)
      + transposes of pooled/clause -> transposed node embeddings as
      bf16 hi/lo pairs xT_h/xT_l [1536, 8*128]
  P2  q/k projections as 3-term bf16 split products (weight pairs are
      host-pre-split); q only for the 64 clause query rows per graph
  P3  per-(graph, head) attention scores (3-term bf16 pair product) +
      masked softmax + head-mean, top-3 row selection via DVE
      max8/match_replace, adjacency transpose
  P4  Ax = adj_sel[clause rows] @ x and GCN matmul in plain bf16
      (post-selection values only need ~1e-3), bias/denominator/relu
      epilogue, transpose back to node-major rows, DMA out.

Precision scheme: everything upstream of the top-3 selection uses bf16
split pairs (v = bf16(v) + bf16(v - bf16(v)), ~17 mantissa bits) with
3-term products (h*h + l*h + h*l) accumulated in fp32 PSUM.  Measured on
the real generator distribution this reproduces the fp32 reference's
top-3 selections exactly (0/4096 flips); plain bf16 or tf32-style
rounding flips selections and fails the 2e-2 gate.  The TensorEngine
runs bf16 at 1 cycle/row vs fp32's 4, so this is ~2.7x less PE time.
"""

import numpy as np
import ml_dtypes

# ---------------- problem constants (hardcoded; kernel must be self-contained)
B_TOTAL = 64          # graphs
L = 128               # nodes per graph
HALF = 64
T = 32                # bert tokens per pair
BD = 768              # BERT_DIM == HIDDEN_DIM
D = 1536              # BD + HIDDEN
HEADS = 8
DK = D // HEADS       # 192
TOPK = 3
N_CORES = 8
G = B_TOTAL // N_CORES            # graphs per core = 8
NAA = G * HALF                    # AA/clause rows per core = 512
DC = D // 128                     # 12 chunks of the D dim
BC = BD // 128                    # 6 chunks of the BERT dim
KCH = (HALF * T) // 128           # 16 selector k-chunks per graph
NODES_PC = G * L                  # 1024 node columns per core
INV_SQRT_DK = 1.0 / float(np.sqrt(DK))

BF16 = ml_dtypes.bfloat16

_STATE = {}


def _split_pair(a):
    """v -> (bf16(v), bf16(v - bf16(v))) host-side split."""
    a = np.ascontiguousarray(np.asarray(a, dtype=np.float32))
    h = a.astype(BF16)
    l = (a - h.astype(np.float32)).astype(BF16)
    return h, l


def _build_nc():
    import concourse.bass as bass
    import concourse.bacc as bacc
    import concourse.mybir as mybir
    import concourse.tile as tile

    f32 = mybir.dt.float32
    bf16 = mybir.dt.bfloat16
    i32 = mybir.dt.int32
    AF = mybir.ActivationFunctionType
    OP = mybir.AluOpType

    nc = bacc.Bacc("TRN2", target_bir_lowering=False, debug=False)

    # ---------------- DRAM parameters (per-core shard shapes)
    bert_h_d = nc.dram_tensor("bert_h", [NAA * T, BD], bf16, kind="ExternalInput")
    bert_l_d = nc.dram_tensor("bert_l", [NAA * T, BD], bf16, kind="ExternalInput")
    pooledT_h_d = nc.dram_tensor("pooledT_h", [BD, NAA], bf16, kind="ExternalInput")
    pooledT_l_d = nc.dram_tensor("pooledT_l", [BD, NAA], bf16, kind="ExternalInput")
    clauseT_h_d = nc.dram_tensor("clauseT_h", [D, NAA], bf16, kind="ExternalInput")
    clauseT_l_d = nc.dram_tensor("clauseT_l", [D, NAA], bf16, kind="ExternalInput")
    blen_d = nc.dram_tensor("batch_aa_bert_length", [NAA], i32, kind="ExternalInput")
    glen_d = nc.dram_tensor("aa_graph_length", [G], i32, kind="ExternalInput")
    dWh_d = nc.dram_tensor("dense_W_h", [BD, BD], bf16, kind="ExternalInput")
    dWl_d = nc.dram_tensor("dense_W_l", [BD, BD], bf16, kind="ExternalInput")
    db_d = nc.dram_tensor("dense_b", [BD], f32, kind="ExternalInput")
    Wqh_d = nc.dram_tensor("Wq_h", [D, D], bf16, kind="ExternalInput")
    Wql_d = nc.dram_tensor("Wq_l", [D, D], bf16, kind="ExternalInput")
    bq_d = nc.dram_tensor("bq", [D], f32, kind="ExternalInput")
    Wkh_d = nc.dram_tensor("Wk_h", [D, D], bf16, kind="ExternalInput")
    Wkl_d = nc.dram_tensor("Wk_l", [D, D], bf16, kind="ExternalInput")
    bk_d = nc.dram_tensor("bk", [D], f32, kind="ExternalInput")
    gW_d = nc.dram_tensor("gcn_W16", [D, D], bf16, kind="ExternalInput")
    gb_d = nc.dram_tensor("gcn_b", [D], f32, kind="ExternalInput")
    out_d = nc.dram_tensor("out", [NAA, D], f32, kind="ExternalOutput")

    # ---------------- inline constants
    # S0[c, r, n] = 1 where n == 4c + r//32  (selector for 4 nodes per k-chunk)
    s0 = np.zeros((KCH, 128, HALF), np.float32)
    for c in range(KCH):
        for r in range(128):
            s0[c, r, 4 * c + r // 32] = 1.0
    s0_d = nc.inline_tensor(
        np.ascontiguousarray(s0.transpose(1, 0, 2)).astype(BF16), name="s0")  # [128,16,64]
    ident_d = nc.inline_tensor(np.eye(128, dtype=np.float32), name="ident")
    ident16_d = nc.inline_tensor(np.eye(128, dtype=np.float32).astype(BF16), name="ident16")
    iota_t_d = nc.inline_tensor(
        (np.arange(128, dtype=np.float32) % T).reshape(128, 1), name="iota_t")
    iota_row_d = nc.inline_tensor(
        np.arange(128, dtype=np.float32).reshape(128, 1), name="iota_row")
    col128_d = nc.inline_tensor(
        np.broadcast_to(np.arange(128, dtype=np.float32), (128, 128)).copy(), name="col128")
    ones1_d = nc.inline_tensor(np.ones((1, 128), np.float32), name="ones1")

    with tile.TileContext(nc) as tc:
        import contextlib
        with contextlib.ExitStack() as ctx:
            cpool = ctx.enter_context(tc.tile_pool(name="const", bufs=1))
            ident = cpool.tile([128, 128], f32)
            nc.sync.dma_start(out=ident[:], in_=ident_d.ap())
            ident16 = cpool.tile([128, 128], bf16)
            nc.sync.dma_start(out=ident16[:], in_=ident16_d.ap())
            iota_row = cpool.tile([128, 1], f32)
            nc.sync.dma_start(out=iota_row[:], in_=iota_row_d.ap())
            col128 = cpool.tile([128, 128], f32)
            nc.sync.dma_start(out=col128[:], in_=col128_d.ap())
            ones1 = cpool.tile([1, 128], f32)
            nc.sync.dma_start(out=ones1[:], in_=ones1_d.ap())

            # ---- biases as [128, chunks] column tiles
            db_t = cpool.tile([128, BC], f32)
            nc.sync.dma_start(out=db_t[:], in_=db_d.ap().rearrange("(c p) -> p c", p=128))
            db32_t = cpool.tile([128, BC], f32)
            nc.vector.tensor_scalar_mul(db32_t[:], db_t[:], float(T))
            bq_t = cpool.tile([128, DC], f32)
            nc.sync.dma_start(out=bq_t[:], in_=bq_d.ap().rearrange("(c p) -> p c", p=128))
            bk_t = cpool.tile([128, DC], f32)
            nc.sync.dma_start(out=bk_t[:], in_=bk_d.ap().rearrange("(c p) -> p c", p=128))
            gb_t = cpool.tile([128, DC], f32)
            nc.sync.dma_start(out=gb_t[:], in_=gb_d.ap().rearrange("(c p) -> p c", p=128))

            # ---- per-graph lengths broadcast across partitions
            glen_row = cpool.tile([1, G], f32)
            glen_i = cpool.tile([1, G], i32)
            nc.sync.dma_start(out=glen_i[:], in_=glen_d.ap().unsqueeze(0))
            nc.vector.tensor_copy(out=glen_row[:], in_=glen_i[:])
            blen_row = cpool.tile([1, NAA], f32)
            blen_i = cpool.tile([1, NAA], i32)
            nc.sync.dma_start(out=blen_i[:], in_=blen_d.ap().unsqueeze(0))
            nc.vector.tensor_copy(out=blen_row[:], in_=blen_i[:])

            with tc.tile_pool(name="bcast_ps", bufs=2, space="PSUM") as bps:
                glen_b = cpool.tile([128, G], f32)       # graph length on every partition
                pb = bps.tile([128, G], f32)
                nc.tensor.matmul(pb[:], lhsT=ones1[:], rhs=glen_row[:], start=True, stop=True)
                nc.vector.tensor_copy(out=glen_b[:], in_=pb[:])
                blen_b = cpool.tile([128, NAA], f32)     # per-pair bert length, bcast
                for h in range(NAA // 512):
                    pb2 = bps.tile([128, 512], f32)
                    nc.tensor.matmul(pb2[:], lhsT=ones1[:], rhs=blen_row[:, h * 512:(h + 1) * 512],
                                     start=True, stop=True)
                    nc.vector.tensor_copy(out=blen_b[:, h * 512:(h + 1) * 512], in_=pb2[:])

            # ---------------- persistent activation tiles (bf16 pairs)
            xT_pool = ctx.enter_context(tc.tile_pool(name="xT", bufs=1))
            xTh = xT_pool.tile([128, DC, NODES_PC], bf16)   # 3.1MB
            xTl = xT_pool.tile([128, DC, NODES_PC], bf16)   # 3.1MB

            def pair_write(dst_h, dst_l, psrc, bias=None):
                """dst_h = bf16(psrc + bias); dst_l = bf16((psrc + bias) - dst_h)."""
                if bias is None:
                    nc.scalar.copy(out=dst_h, in_=psrc)
                    nc.vector.tensor_tensor(out=dst_l, in0=psrc, in1=dst_h, op=OP.subtract)
                else:
                    nc.scalar.activation(out=dst_h, in_=psrc, func=AF.Identity,
                                         bias=bias, scale=1.0)
                    nc.vector.scalar_tensor_tensor(out=dst_l, in0=psrc, scalar=bias,
                                                   in1=dst_h, op0=OP.add, op1=OP.subtract)

            # ---- fill pooled/clause parts of xT straight from host-transposed
            #      bf16 pairs (no device transposes, no upstream deps)
            xTh5 = xTh[:].rearrange("p c (g l) -> p c g l", l=L)
            xTl5 = xTl[:].rearrange("p c (g l) -> p c g l", l=L)
            for dcH in range(BC):
                rs = slice(dcH * 128, (dcH + 1) * 128)
                nc.sync.dma_start(
                    out=xTh5[:, BC + dcH, :, 0:HALF],
                    in_=pooledT_h_d.ap()[rs, :].rearrange("p (g n) -> p g n", n=HALF))
                nc.sync.dma_start(
                    out=xTl5[:, BC + dcH, :, 0:HALF],
                    in_=pooledT_l_d.ap()[rs, :].rearrange("p (g n) -> p g n", n=HALF))
            for dcH in range(DC):
                rs = slice(dcH * 128, (dcH + 1) * 128)
                nc.sync.dma_start(
                    out=xTh5[:, dcH, :, HALF:L],
                    in_=clauseT_h_d.ap()[rs, :].rearrange("p (g n) -> p g n", n=HALF))
                nc.sync.dma_start(
                    out=xTl5[:, dcH, :, HALF:L],
                    in_=clauseT_l_d.ap()[rs, :].rearrange("p (g n) -> p g n", n=HALF))

            # =========================================================
            # P1+P2 interleaved by graph-halves: selector -> dense -> q/k
            # =========================================================
            xTh4 = xTh[:].rearrange("p c (g l) -> p c g l", l=L)
            xTl4 = xTl[:].rearrange("p c (g l) -> p c g l", l=L)
            adjT_pool = ctx.enter_context(tc.tile_pool(name="adjT", bufs=1))
            adjT_all = adjT_pool.tile([128, G, HALF], bf16)    # adj_sel^T per graph
            denr_row = cpool.tile([1, NAA], f32)               # 1/denom per clause row
            p3w = ctx.enter_context(tc.tile_pool(name="p3w", bufs=2))
            gw_pool = ctx.enter_context(tc.tile_pool(name="gw", bufs=2))
            GH = G // 2                                        # graphs per half
            with tc.tile_pool(name="qkT", bufs=1) as qkT_pool:
                qT = qkT_pool.tile([128, DC, G * HALF], f32)
                kT = qkT_pool.tile([128, DC, NODES_PC], f32)
                DOUT_BLK = 2   # dout chunks per weight block (256 cols)
                with (
                    tc.tile_pool(name="tokT", bufs=1) as tokT_pool,
                    tc.tile_pool(name="p1sb", bufs=4) as p1sb,
                    tc.tile_pool(name="wblk", bufs=2) as w_pool,
                    tc.tile_pool(name="mps", bufs=1, space="PSUM") as mps,
                ):
                    s0_t = p1sb.tile([128, KCH, HALF], bf16, tag="s0t", bufs=1)
                    nc.sync.dma_start(out=s0_t[:], in_=s0_d.ap())
                    iota_t = p1sb.tile([128, 1], f32, tag="iota_t", bufs=1)
                    nc.sync.dma_start(out=iota_t[:], in_=iota_t_d.ap())
                    def p1_graph(g, tokTh_hf, tokTl_hf):
                        gl = g % GH
                        lt = p1sb.tile([128, HALF], bf16, tag="lt", name="lt")
                        nc.vector.tensor_tensor(
                            out=lt[:], in0=blen_b[:, g * HALF:(g + 1) * HALF],
                            in1=iota_t[:].to_broadcast([128, HALF]), op=OP.is_gt)
                        ptok = mps.tile([64, BD], f32, tag="ptok", name="ptok", bufs=1)
                        for c in range(KCH):
                            bh_t = p1sb.tile([128, BD], bf16, tag="bh", name="bh", bufs=3)
                            bl_t = p1sb.tile([128, BD], bf16, tag="bl", name="bl", bufs=3)
                            r0 = g * (HALF * T) + c * 128
                            nc.sync.dma_start(out=bh_t[:], in_=bert_h_d.ap()[r0:r0 + 128, :])
                            nc.sync.dma_start(out=bl_t[:], in_=bert_l_d.ap()[r0:r0 + 128, :])
                            sm = p1sb.tile([128, HALF], bf16, tag="sm", name="sm")
                            nc.vector.tensor_tensor(out=sm[:], in0=s0_t[:, c, :], in1=lt[:],
                                                    op=OP.mult)
                            first, last = (c == 0), (c == KCH - 1)
                            nc.tensor.matmul(ptok[:, 0:512], lhsT=sm[:], rhs=bh_t[:, 0:512],
                                             start=first, stop=False)
                            nc.tensor.matmul(ptok[:, 0:512], lhsT=sm[:], rhs=bl_t[:, 0:512],
                                             start=False, stop=last)
                            nc.tensor.matmul(ptok[:, 512:768], lhsT=sm[:], rhs=bh_t[:, 512:768],
                                             start=first, stop=False)
                            nc.tensor.matmul(ptok[:, 512:768], lhsT=sm[:], rhs=bl_t[:, 512:768],
                                             start=False, stop=last)
                        tok_ng = p1sb.tile([64, BD], f32, tag="tokng", name="tokng", bufs=1)
                        nc.scalar.copy(out=tok_ng[:], in_=ptok[:])
                        for dcH in range(BC):
                            ptr = mps.tile([128, HALF], f32, tag="ptr", name="ptr", bufs=2)
                            nc.tensor.matmul(ptr[:], lhsT=tok_ng[:, dcH * 128:(dcH + 1) * 128],
                                             rhs=ident[0:64, 0:64], start=True, stop=True,
                                             is_transpose=True)
                            pair_write(tokTh_hf[:, dcH, gl * HALF:(gl + 1) * HALF],
                                       tokTl_hf[:, dcH, gl * HALF:(gl + 1) * HALF], ptr[:])

                    def dense_half(hf, tokTh_hf, tokTl_hf):
                        for dco in range(BC):
                            csl = slice(dco * 128, (dco + 1) * 128)
                            dwh = w_pool.tile([128, BC, 128], bf16, tag="wh", name="dwh")
                            dwl = w_pool.tile([128, BC, 128], bf16, tag="wl", name="dwl")
                            nc.sync.dma_start(
                                out=dwh[:],
                                in_=dWh_d.ap().rearrange("(c p) e -> p c e", p=128)[:, :, csl])
                            nc.sync.dma_start(
                                out=dwl[:],
                                in_=dWl_d.ap().rearrange("(c p) e -> p c e", p=128)[:, :, csl])
                            pd = mps.tile([128, GH * HALF], f32, tag="pqk", name="pd", bufs=3)
                            for dci in range(BC):
                                nc.tensor.matmul(pd[:], lhsT=dwh[:, dci, :],
                                                 rhs=tokTh_hf[:, dci, :],
                                                 start=(dci == 0), stop=False)
                                nc.tensor.matmul(pd[:], lhsT=dwh[:, dci, :],
                                                 rhs=tokTl_hf[:, dci, :], start=False, stop=False)
                                nc.tensor.matmul(pd[:], lhsT=dwl[:, dci, :],
                                                 rhs=tokTh_hf[:, dci, :], start=False,
                                                 stop=(dci == BC - 1))
                            gsl = slice(hf * GH, (hf + 1) * GH)
                            dsth = xTh4[:, dco, gsl, 0:HALF]
                            dstl = xTl4[:, dco, gsl, 0:HALF]
                            psrc = pd[:].rearrange("p (g h) -> p g h", h=HALF)
                            pair_write(dsth, dstl, psrc, bias=db32_t[:, dco:dco + 1])

                    def load_wblk(Wh_d_, Wl_d_, blk):
                        csl = slice(blk * DOUT_BLK * 128, (blk + 1) * DOUT_BLK * 128)
                        wh = w_pool.tile([128, DC, DOUT_BLK * 128], bf16, tag="wh", name="wh")
                        wl = w_pool.tile([128, DC, DOUT_BLK * 128], bf16, tag="wl", name="wl")
                        nc.sync.dma_start(
                            out=wh[:], in_=Wh_d_.ap().rearrange("(c p) e -> p c e", p=128)[:, :, csl])
                        nc.sync.dma_start(
                            out=wl[:], in_=Wl_d_.ap().rearrange("(c p) e -> p c e", p=128)[:, :, csl])
                        return wh, wl

                    def k_half(hf):
                        nsl = slice(hf * GH * L, (hf + 1) * GH * L)
                        for blk in range(DC // DOUT_BLK):
                            wh, wl = load_wblk(Wkh_d, Wkl_d, blk)
                            for j in range(DOUT_BLK):
                                dco = blk * DOUT_BLK + j
                                jsl = slice(j * 128, (j + 1) * 128)
                                pk = mps.tile([128, GH * L], f32, tag="pqk", name="pk", bufs=3)
                                for dci in range(DC):
                                    nc.tensor.matmul(pk[:], lhsT=wh[:, dci, jsl],
                                                     rhs=xTh[:, dci, nsl],
                                                     start=(dci == 0), stop=False)
                                    nc.tensor.matmul(pk[:], lhsT=wh[:, dci, jsl],
                                                     rhs=xTl[:, dci, nsl],
                                                     start=False, stop=False)
                                    nc.tensor.matmul(pk[:], lhsT=wl[:, dci, jsl],
                                                     rhs=xTh[:, dci, nsl],
                                                     start=False, stop=(dci == DC - 1))
                                nc.scalar.activation(out=kT[:, dco, nsl], in_=pk[:],
                                                     func=AF.Identity,
                                                     bias=bk_t[:, dco:dco + 1], scale=1.0)

                    def q_full():
                        for blk in range(DC // DOUT_BLK):
                            wh, wl = load_wblk(Wqh_d, Wql_d, blk)
                            for j in range(DOUT_BLK):
                                dco = blk * DOUT_BLK + j
                                jsl = slice(j * 128, (j + 1) * 128)
                                pq = mps.tile([128, G * HALF], f32, tag="pqk", name="pq", bufs=3)
                                for dci in range(DC):
                                    nc.tensor.matmul(pq[:], lhsT=wh[:, dci, jsl],
                                                     rhs=xTh4[:, dci, :, HALF:L],
                                                     start=(dci == 0), stop=False)
                                    nc.tensor.matmul(pq[:], lhsT=wh[:, dci, jsl],
                                                     rhs=xTl4[:, dci, :, HALF:L],
                                                     start=False, stop=False)
                                    nc.tensor.matmul(pq[:], lhsT=wl[:, dci, jsl],
                                                     rhs=xTh4[:, dci, :, HALF:L],
                                                     start=False, stop=(dci == DC - 1))
                                nc.scalar.activation(out=qT[:, dco, :], in_=pq[:],
                                                     func=AF.Identity,
                                                     bias=bq_t[:, dco:dco + 1], scale=1.0)

                    for hf in range(2):
                        tokTh_hf = tokT_pool.tile([128, BC, GH * HALF], bf16,
                                                  tag="tokTh", name="tokTh")
                        tokTl_hf = tokT_pool.tile([128, BC, GH * HALF], bf16,
                                                  tag="tokTl", name="tokTl")
                        for g in range(hf * GH, (hf + 1) * GH):
                            p1_graph(g, tokTh_hf, tokTl_hf)
                        dense_half(hf, tokTh_hf, tokTl_hf)
                        k_half(hf)
                    q_full()

                # =========================================================
                # P3: attention + top-3 per graph (clause query rows only)
                # =========================================================
                axT = adjT_pool.tile([128, DC, NAA], bf16)   # (A @ x)^T, clause rows
                with (
                    tc.tile_pool(name="att", bufs=2) as att_pool,
                    tc.tile_pool(name="attsc", bufs=2) as attsc_pool,
                    tc.tile_pool(name="p3ps", bufs=2, space="PSUM") as p3ps,
                ):
                    for g in range(G):
                        colmask = att_pool.tile([128, L], f32, tag="colmask")
                        nc.vector.tensor_scalar(
                            out=colmask[:], in0=col128[:], scalar1=glen_b[:, g:g + 1],
                            scalar2=-1e9, op0=OP.is_ge, op1=OP.mult)
                        vrow = attsc_pool.tile([128, 1], f32, tag="vrow")
                        nc.vector.tensor_scalar(
                            out=vrow[:], in0=iota_row[:], scalar1=glen_b[:, g:g + 1],
                            scalar2=1.0 / HEADS, op0=OP.is_lt, op1=OP.mult)
                        adj = att_pool.tile([64, L], f32, tag="adj")
                        for h in range(HEADS):
                            ps = p3ps.tile([64, L], f32, tag="pscore")
                            r0 = h * DK
                            chunks = [(ra, rb) for (ra, rb) in
                                      ((r0, min(r0 + DK, (r0 // 128 + 1) * 128)),
                                       ((r0 // 128 + 1) * 128, r0 + DK)) if rb > ra]
                            for ci, (ra, rb) in enumerate(chunks):
                                tI, p0 = ra // 128, ra % 128
                                p1_ = p0 + (rb - ra)
                                qs = slice(g * HALF, (g + 1) * HALF)
                                ks = slice(g * L, (g + 1) * L)
                                nc.tensor.matmul(ps[:], lhsT=qT[p0:p1_, tI, qs],
                                                 rhs=kT[p0:p1_, tI, ks],
                                                 start=(ci == 0), stop=(ci == len(chunks) - 1))
                            # mask invalid key columns; softmax over keys of s/sqrt(dk)
                            nc.vector.tensor_tensor(out=ps[:], in0=ps[:], in1=colmask[0:64, :],
                                                    op=OP.add)
                            negmax = attsc_pool.tile([64, 1], f32, tag="negmax")
                            nc.vector.reduce_max(out=negmax[:], in_=ps[:],
                                                 axis=mybir.AxisListType.X, negate=True)
                            nms = attsc_pool.tile([64, 1], f32, tag="nms")
                            nc.vector.tensor_scalar_mul(nms[:], negmax[:], INV_SQRT_DK)
                            exph = att_pool.tile([64, L], f32, tag="exph")
                            sumexp = attsc_pool.tile([64, 1], f32, tag="sumexp")
                            nc.scalar.activation(out=exph[:], in_=ps[:], func=AF.Exp,
                                                 bias=nms[:], scale=INV_SQRT_DK,
                                                 accum_out=sumexp[:])
                            recip = attsc_pool.tile([64, 1], f32, tag="recip")
                            nc.vector.reciprocal(out=recip[:], in_=sumexp[:])
                            if h == 0:
                                nc.vector.tensor_scalar(out=adj[:], in0=exph[:], scalar1=recip[:],
                                                        scalar2=None, op0=OP.mult)
                            else:
                                nc.vector.scalar_tensor_tensor(
                                    out=adj[:], in0=exph[:], scalar=recip[:], in1=adj[:],
                                    op0=OP.mult, op1=OP.add)
                        nc.vector.tensor_scalar(out=adj[:], in0=adj[:], scalar1=vrow[64:128, :],
                                                scalar2=None, op0=OP.mult)
                        # top-3 selection
                        top8 = attsc_pool.tile([64, 8], f32, tag="top8")
                        nc.vector.max(out=top8[:], in_=adj[:])
                        nc.vector.memset(top8[:, TOPK:8], 0.0)
                        zapped = att_pool.tile([64, L], f32, tag="zapped")
                        nc.vector.match_replace(out=zapped[:], in_to_replace=top8[:],
                                                in_values=adj[:], imm_value=0.0)
                        adjsel = att_pool.tile([64, L], f32, tag="adjsel")
                        denom = attsc_pool.tile([64, 1], f32, tag="denom")
                        nc.vector.tensor_tensor(out=adjsel[:], in0=adj[:], in1=zapped[:],
                                                op=OP.subtract)
                        nc.vector.reduce_sum(out=denom[:], in_=adjsel[:],
                                             axis=mybir.AxisListType.X)
                        nc.vector.tensor_scalar_add(denom[:], denom[:], 1.0)
                        recip_d = attsc_pool.tile([64, 1], f32, tag="recipd")
                        nc.vector.reciprocal(out=recip_d[:], in_=denom[:])
                        # adjT [128 nodes, 64 clause rows] in bf16
                        pat = p3ps.tile([128, 64], f32, tag="padjT")
                        nc.tensor.matmul(pat[:], lhsT=adjsel[:], rhs=ident[0:64, 0:64],
                                         start=True, stop=True, is_transpose=True)
                        nc.vector.tensor_copy(out=adjT_all[:, g, :], in_=pat[:])
                        # 1/denom -> row vector [1, 64]
                        prd = p3ps.tile([1, 64], f32, tag="pdenr", bufs=1)
                        nc.tensor.matmul(prd[:], lhsT=recip_d[:], rhs=ident[0:64, 0:64],
                                         start=True, stop=True, is_transpose=True)
                        nc.vector.tensor_copy(out=denr_row[:, g * HALF:(g + 1) * HALF], in_=prd[:])
                        # ---- Ax for this graph (plain bf16, post-selection)
                        xg = p3w.tile([128, D], bf16, tag="xg", name="xg")
                        for dcH in range(DC):
                            pxt = p3ps.tile([128, 128], bf16, tag="pxg", bufs=1)
                            nc.tensor.matmul(pxt[:], lhsT=xTh[:, dcH, g * L:(g + 1) * L],
                                             rhs=ident16[:], start=True, stop=True,
                                             is_transpose=True)
                            nc.scalar.copy(out=xg[:, dcH * 128:(dcH + 1) * 128], in_=pxt[:])
                        ax = p3w.tile([64, D], bf16, tag="ax", name="ax")
                        for nh in range(3):
                            pax = p3ps.tile([64, 512], f32, tag="pax", bufs=1)
                            nc.tensor.matmul(pax[:], lhsT=adjT_all[:, g, :],
                                             rhs=xg[:, nh * 512:(nh + 1) * 512],
                                             start=True, stop=True)
                            nc.scalar.copy(out=ax[:, nh * 512:(nh + 1) * 512], in_=pax[:])
                        for dcH in range(DC):
                            paxt = p3ps.tile([128, 64], bf16, tag="paxt", bufs=1)
                            nc.tensor.matmul(paxt[:], lhsT=ax[:, dcH * 128:(dcH + 1) * 128],
                                             rhs=ident16[0:64, 0:64], start=True, stop=True,
                                             is_transpose=True)
                            nc.scalar.copy(out=axT[:, dcH, g * HALF:(g + 1) * HALF], in_=paxt[:])

            # =========================================================
            # P4: GCN (bf16, graph-half blocked), epilogue, output
            # =========================================================
            with (
                tc.tile_pool(name="ostage", bufs=1) as ost_pool,
                tc.tile_pool(name="gtmp", bufs=3) as gtmp_pool,
                tc.tile_pool(name="p4b", bufs=3, space="PSUM") as p4bps,
            ):
                HN = NAA // 2
                denrb = gtmp_pool.tile([128, NAA], f32, tag="denrb", bufs=1)
                for hf2 in range(2):
                    cs = slice(hf2 * HN, (hf2 + 1) * HN)
                    pb3 = p4bps.tile([128, HN], f32, tag="pgcn")
                    nc.tensor.matmul(pb3[:], lhsT=ones1[:], rhs=denr_row[:, cs],
                                     start=True, stop=True)
                    nc.vector.tensor_copy(out=denrb[:, cs], in_=pb3[:])
                ostage = [ost_pool.tile([64, D], f32, tag=f"ost{g}", name=f"ost{g}")
                          for g in range(G)]
                GBLK = 1
                for blk in range(DC // GBLK):
                    csl = slice(blk * GBLK * 128, (blk + 1) * GBLK * 128)
                    gwt = gw_pool.tile([128, DC, GBLK * 128], bf16, tag="gw", name="gwt")
                    nc.sync.dma_start(
                        out=gwt[:], in_=gW_d.ap().rearrange("(c p) e -> p c e", p=128)[:, :, csl])
                    for j in range(GBLK):
                        dco = blk * GBLK + j
                        for hf2 in range(2):
                            cs = slice(hf2 * HN, (hf2 + 1) * HN)
                            pg = p4bps.tile([128, HN], f32, tag="pgcn")
                            for dci in range(DC):
                                nc.tensor.matmul(pg[:], lhsT=gwt[:, dci, j * 128:(j + 1) * 128],
                                                 rhs=axT[:, dci, cs],
                                                 start=(dci == 0), stop=(dci == DC - 1))
                            trel = gtmp_pool.tile([128, HN], f32, tag="trel")
                            nc.scalar.activation(out=trel[:], in_=pg[:], func=AF.Relu,
                                                 bias=gb_t[:, dco:dco + 1], scale=1.0)
                            trel16 = gtmp_pool.tile([128, HN], bf16, tag="trel16")
                            nc.vector.tensor_tensor(out=trel16[:], in0=trel[:],
                                                    in1=denrb[:, cs], op=OP.mult)
                            for gl in range(G // 2):
                                g = hf2 * (G // 2) + gl
                                po = p4bps.tile([64, 128], bf16, tag="pout")
                                nc.tensor.matmul(po[:],
                                                 lhsT=trel16[:, gl * HALF:(gl + 1) * HALF],
                                                 rhs=ident16[:], start=True, stop=True,
                                                 is_transpose=True)
                                nc.scalar.copy(out=ostage[g][:, dco * 128:(dco + 1) * 128],
                                               in_=po[:])
                for g in range(G):
                    nc.sync.dma_start(out=out_d.ap()[g * HALF:(g + 1) * HALF, :],
                                      in_=ostage[g][:])

    nc.compile()
    return nc


def _get_nc():
    if "nc" not in _STATE:
        _STATE["nc"] = _build_nc()
    return _STATE["nc"]


def _shard_inputs(inputs):
    """Split full inputs into 8 per-core maps (data-parallel over graphs),
    pre-splitting bf16 hi/lo pairs for BERT and the projection weights."""
    bert = np.ascontiguousarray(np.asarray(inputs["inner_bert_out"], dtype=np.float32))
    bert_h, bert_l = _split_pair(bert.reshape(B_TOTAL * HALF * T, BD))
    pooled = np.asarray(inputs["inner_pooled_out"], dtype=np.float32)
    clause = np.asarray(inputs["clause_output"], dtype=np.float32)
    blen = np.ascontiguousarray(np.asarray(inputs["batch_aa_bert_length"], dtype=np.int32))
    glen = np.ascontiguousarray(np.asarray(inputs["aa_graph_length"], dtype=np.int32))
    dWh, dWl = _split_pair(inputs["dense_W"])
    Wqh, Wql = _split_pair(inputs["Wq"])
    Wkh, Wkl = _split_pair(inputs["Wk"])
    gW16 = np.asarray(inputs["gcn_W"], dtype=np.float32).astype(BF16)
    reps = {
        "dense_W_h": dWh, "dense_W_l": dWl,
        "dense_b": np.asarray(inputs["dense_b"], np.float32),
        "Wq_h": Wqh, "Wq_l": Wql, "bq": np.asarray(inputs["bq"], np.float32),
        "Wk_h": Wkh, "Wk_l": Wkl, "bk": np.asarray(inputs["bk"], np.float32),
        "gcn_W16": gW16, "gcn_b": np.asarray(inputs["gcn_b"], np.float32),
    }
    in_maps = []
    rt = HALF * T
    for c in range(N_CORES):
        r0, r1 = c * NAA, (c + 1) * NAA
        pTh, pTl = _split_pair(np.ascontiguousarray(pooled[r0:r1].T))
        cTh, cTl = _split_pair(np.ascontiguousarray(clause[r0:r1].T))
        m = {
            "bert_h": bert_h[r0 * T:r1 * T],
            "bert_l": bert_l[r0 * T:r1 * T],
            "pooledT_h": pTh, "pooledT_l": pTl,
            "clauseT_h": cTh, "clauseT_l": cTl,
            "batch_aa_bert_length": blen[r0:r1],
            "aa_graph_length": glen[c * G:(c + 1) * G],
        }
        m.update(reps)
        in_maps.append(m)
    return in_maps


def kernel(**inputs) -> np.ndarray:
    from concourse.bass_utils import run_bass_kernel_spmd

    nc = _get_nc()
    in_maps = _shard_inputs(inputs)
    res = run_bass_kernel_spmd(nc, in_maps, core_ids=list(range(N_CORES)))
    return np.concatenate([res.results[c]["out"] for c in range(N_CORES)], axis=0)


# revision 45
# speedup vs baseline: 1.0526x; 1.0526x over previous
"""Trainium2 Bass kernel for nn_AA_encoder (gnn_message_passing).

Data-parallel over the B=64 graph dimension: 8 graphs per NeuronCore on a
single TRN2 chip (8 cores).  Per-core pipeline:

  P1  masked BERT token-sum (PE selector matmuls over host-pre-split
      bf16 hi/lo BERT pairs) -> tok_sum, transposed to tok_sumT pairs
  P1b dense projection (3-term bf16 split product, batched over graphs)
      + transposes of pooled/clause -> transposed node embeddings as
      bf16 hi/lo pairs xT_h/xT_l [1536, 8*128]
  P2  q/k projections as 3-term bf16 split products (weight pairs are
      host-pre-split); q only for the 64 clause query rows per graph
  P3  per-(graph, head) attention scores (3-term bf16 pair product) +
      masked softmax + head-mean, top-3 row selection via DVE
      max8/match_replace, adjacency transpose
  P4  Ax = adj_sel[clause rows] @ x and GCN matmul in plain bf16
      (post-selection values only need ~1e-3), bias/denominator/relu
      epilogue, transpose back to node-major rows, DMA out.

Precision scheme: everything upstream of the top-3 selection uses bf16
split pairs (v = bf16(v) + bf16(v - bf16(v)), ~17 mantissa bits) with
3-term products (h*h + l*h + h*l) accumulated in fp32 PSUM.  Measured on
the real generator distribution this reproduces the fp32 reference's
top-3 selections exactly (0/4096 flips); plain bf16 or tf32-style
rounding flips selections and fails the 2e-2 gate.  The TensorEngine
runs bf16 at 1 cycle/row vs fp32's 4, so this is ~2.7x less PE time.
"""

import numpy as np
import ml_dtypes

# ---------------- problem constants (hardcoded; kernel must be self-contained)
B_TOTAL = 64          # graphs
L = 128               # nodes per graph
HALF = 64
T = 32                # bert tokens per pair
BD = 768              # BERT_DIM == HIDDEN_DIM
D = 1536              # BD + HIDDEN
HEADS = 8
DK = D // HEADS       # 192
TOPK = 3
N_CORES = 8
G = B_TOTAL // N_CORES            # graphs per core = 8
NAA = G * HALF                    # AA/clause rows per core = 512
DC = D // 128                     # 12 chunks of the D dim
BC = BD // 128                    # 6 chunks of the BERT dim
KCH = (HALF * T) // 128           # 16 selector k-chunks per graph
NODES_PC = G * L                  # 1024 node columns per core
INV_SQRT_DK = 1.0 / float(np.sqrt(DK))

BF16 = ml_dtypes.bfloat16

_STATE = {}


def _split_pair(a):
    """v -> (bf16(v), bf16(v - bf16(v))) host-side split."""
    a = np.ascontiguousarray(np.asarray(a, dtype=np.float32))
    h = a.astype(BF16)
    l = (a - h.astype(np.float32)).astype(BF16)
    return h, l


def _build_nc():
    import concourse.bass as bass
    import concourse.bacc as bacc
    import concourse.mybir as mybir
    import concourse.tile as tile

    f32 = mybir.dt.float32
    bf16 = mybir.dt.bfloat16
    i32 = mybir.dt.int32
    AF = mybir.ActivationFunctionType
    OP = mybir.AluOpType

    nc = bacc.Bacc("TRN2", target_bir_lowering=False, debug=False)

    # ---------------- DRAM parameters (per-core shard shapes)
    bert_h_d = nc.dram_tensor("bert_h", [NAA * T, BD], bf16, kind="ExternalInput")
    bert_l_d = nc.dram_tensor("bert_l", [NAA * T, BD], bf16, kind="ExternalInput")
    pooledT_h_d = nc.dram_tensor("pooledT_h", [BD, NAA], bf16, kind="ExternalInput")
    pooledT_l_d = nc.dram_tensor("pooledT_l", [BD, NAA], bf16, kind="ExternalInput")
    clauseT_h_d = nc.dram_tensor("clauseT_h", [D, NAA], bf16, kind="ExternalInput")
    clauseT_l_d = nc.dram_tensor("clauseT_l", [D, NAA], bf16, kind="ExternalInput")
    blen_d = nc.dram_tensor("batch_aa_bert_length", [NAA], i32, kind="ExternalInput")
    glen_d = nc.dram_tensor("aa_graph_length", [G], i32, kind="ExternalInput")
    dWh_d = nc.dram_tensor("dense_W_h", [BD, BD], bf16, kind="ExternalInput")
    dWl_d = nc.dram_tensor("dense_W_l", [BD, BD], bf16, kind="ExternalInput")
    db_d = nc.dram_tensor("dense_b", [BD], f32, kind="ExternalInput")
    Wqh_d = nc.dram_tensor("Wq_h", [D, D], bf16, kind="ExternalInput")
    Wql_d = nc.dram_tensor("Wq_l", [D, D], bf16, kind="ExternalInput")
    bq_d = nc.dram_tensor("bq", [D], f32, kind="ExternalInput")
    Wkh_d = nc.dram_tensor("Wk_h", [D, D], bf16, kind="ExternalInput")
    Wkl_d = nc.dram_tensor("Wk_l", [D, D], bf16, kind="ExternalInput")
    bk_d = nc.dram_tensor("bk", [D], f32, kind="ExternalInput")
    gW_d = nc.dram_tensor("gcn_W16", [D, D], bf16, kind="ExternalInput")
    gb_d = nc.dram_tensor("gcn_b", [D], f32, kind="ExternalInput")
    out_d = nc.dram_tensor("out", [NAA, D], f32, kind="ExternalOutput")

    # ---------------- inline constants
    # S0[c, r, n] = 1 where n == 4c + r//32  (selector for 4 nodes per k-chunk)
    s0 = np.zeros((KCH, 128, HALF), np.float32)
    for c in range(KCH):
        for r in range(128):
            s0[c, r, 4 * c + r // 32] = 1.0
    s0_d = nc.inline_tensor(
        np.ascontiguousarray(s0.transpose(1, 0, 2)).astype(BF16), name="s0")  # [128,16,64]
    ident_d = nc.inline_tensor(np.eye(128, dtype=np.float32), name="ident")
    ident16_d = nc.inline_tensor(np.eye(128, dtype=np.float32).astype(BF16), name="ident16")
    iota_t_d = nc.inline_tensor(
        (np.arange(128, dtype=np.float32) % T).reshape(128, 1), name="iota_t")
    iota_row_d = nc.inline_tensor(
        np.arange(128, dtype=np.float32).reshape(128, 1), name="iota_row")
    col128_d = nc.inline_tensor(
        np.broadcast_to(np.arange(128, dtype=np.float32), (128, 128)).copy(), name="col128")
    ones1_d = nc.inline_tensor(np.ones((1, 128), np.float32), name="ones1")

    with tile.TileContext(nc) as tc:
        import contextlib
        with contextlib.ExitStack() as ctx:
            cpool = ctx.enter_context(tc.tile_pool(name="const", bufs=1))
            ident = cpool.tile([128, 128], f32)
            nc.sync.dma_start(out=ident[:], in_=ident_d.ap())
            ident16 = cpool.tile([128, 128], bf16)
            nc.sync.dma_start(out=ident16[:], in_=ident16_d.ap())
            iota_row = cpool.tile([128, 1], f32)
            nc.sync.dma_start(out=iota_row[:], in_=iota_row_d.ap())
            col128 = cpool.tile([128, 128], f32)
            nc.sync.dma_start(out=col128[:], in_=col128_d.ap())
            ones1 = cpool.tile([1, 128], f32)
            nc.sync.dma_start(out=ones1[:], in_=ones1_d.ap())

            # ---- biases as [128, chunks] column tiles
            db_t = cpool.tile([128, BC], f32)
            nc.sync.dma_start(out=db_t[:], in_=db_d.ap().rearrange("(c p) -> p c", p=128))
            db32_t = cpool.tile([128, BC], f32)
            nc.vector.tensor_scalar_mul(db32_t[:], db_t[:], float(T))
            bq_t = cpool.tile([128, DC], f32)
            nc.sync.dma_start(out=bq_t[:], in_=bq_d.ap().rearrange("(c p) -> p c", p=128))
            bk_t = cpool.tile([128, DC], f32)
            nc.sync.dma_start(out=bk_t[:], in_=bk_d.ap().rearrange("(c p) -> p c", p=128))
            gb_t = cpool.tile([128, DC], f32)
            nc.sync.dma_start(out=gb_t[:], in_=gb_d.ap().rearrange("(c p) -> p c", p=128))

            # ---- per-graph lengths broadcast across partitions
            glen_row = cpool.tile([1, G], f32)
            glen_i = cpool.tile([1, G], i32)
            nc.sync.dma_start(out=glen_i[:], in_=glen_d.ap().unsqueeze(0))
            nc.vector.tensor_copy(out=glen_row[:], in_=glen_i[:])
            blen_row = cpool.tile([1, NAA], f32)
            blen_i = cpool.tile([1, NAA], i32)
            nc.sync.dma_start(out=blen_i[:], in_=blen_d.ap().unsqueeze(0))
            nc.vector.tensor_copy(out=blen_row[:], in_=blen_i[:])

            with tc.tile_pool(name="bcast_ps", bufs=2, space="PSUM") as bps:
                glen_b = cpool.tile([128, G], f32)       # graph length on every partition
                pb = bps.tile([128, G], f32)
                nc.tensor.matmul(pb[:], lhsT=ones1[:], rhs=glen_row[:], start=True, stop=True)
                nc.vector.tensor_copy(out=glen_b[:], in_=pb[:])
                blen_b = cpool.tile([128, NAA], f32)     # per-pair bert length, bcast
                for h in range(NAA // 512):
                    pb2 = bps.tile([128, 512], f32)
                    nc.tensor.matmul(pb2[:], lhsT=ones1[:], rhs=blen_row[:, h * 512:(h + 1) * 512],
                                     start=True, stop=True)
                    nc.vector.tensor_copy(out=blen_b[:, h * 512:(h + 1) * 512], in_=pb2[:])

            # ---------------- persistent activation tiles (bf16 pairs)
            xT_pool = ctx.enter_context(tc.tile_pool(name="xT", bufs=1))
            xTh = xT_pool.tile([128, DC, NODES_PC], bf16)   # 3.1MB
            xTl = xT_pool.tile([128, DC, NODES_PC], bf16)   # 3.1MB

            def pair_write(dst_h, dst_l, psrc, bias=None):
                """dst_h = bf16(psrc + bias); dst_l = bf16((psrc + bias) - dst_h)."""
                if bias is None:
                    nc.scalar.copy(out=dst_h, in_=psrc)
                    nc.vector.tensor_tensor(out=dst_l, in0=psrc, in1=dst_h, op=OP.subtract)
                else:
                    nc.scalar.activation(out=dst_h, in_=psrc, func=AF.Identity,
                                         bias=bias, scale=1.0)
                    nc.vector.scalar_tensor_tensor(out=dst_l, in0=psrc, scalar=bias,
                                                   in1=dst_h, op0=OP.add, op1=OP.subtract)

            # ---- fill pooled/clause parts of xT straight from host-transposed
            #      bf16 pairs (no device transposes, no upstream deps)
            xTh5 = xTh[:].rearrange("p c (g l) -> p c g l", l=L)
            xTl5 = xTl[:].rearrange("p c (g l) -> p c g l", l=L)
            for dcH in range(BC):
                rs = slice(dcH * 128, (dcH + 1) * 128)
                nc.sync.dma_start(
                    out=xTh5[:, BC + dcH, :, 0:HALF],
                    in_=pooledT_h_d.ap()[rs, :].rearrange("p (g n) -> p g n", n=HALF))
                nc.sync.dma_start(
                    out=xTl5[:, BC + dcH, :, 0:HALF],
                    in_=pooledT_l_d.ap()[rs, :].rearrange("p (g n) -> p g n", n=HALF))
            for dcH in range(DC):
                rs = slice(dcH * 128, (dcH + 1) * 128)
                nc.sync.dma_start(
                    out=xTh5[:, dcH, :, HALF:L],
                    in_=clauseT_h_d.ap()[rs, :].rearrange("p (g n) -> p g n", n=HALF))
                nc.sync.dma_start(
                    out=xTl5[:, dcH, :, HALF:L],
                    in_=clauseT_l_d.ap()[rs, :].rearrange("p (g n) -> p g n", n=HALF))

            # =========================================================
            # P1+P2 interleaved by graph-halves: selector -> dense -> q/k
            # =========================================================
            xTh4 = xTh[:].rearrange("p c (g l) -> p c g l", l=L)
            xTl4 = xTl[:].rearrange("p c (g l) -> p c g l", l=L)
            adjT_pool = ctx.enter_context(tc.tile_pool(name="adjT", bufs=1))
            adjT_all = adjT_pool.tile([128, G, HALF], bf16)    # adj_sel^T per graph
            denr_row = cpool.tile([1, NAA], f32)               # 1/denom per clause row
            p3w = ctx.enter_context(tc.tile_pool(name="p3w", bufs=2))
            GH = G // 2                                        # graphs per half
            with tc.tile_pool(name="qkT", bufs=1) as qkT_pool:
                qTh = qkT_pool.tile([128, DC, G * HALF], bf16)
                qTl = qkT_pool.tile([128, DC, G * HALF], bf16)
                kTh = qkT_pool.tile([128, DC, NODES_PC], bf16)
                kTl = qkT_pool.tile([128, DC, NODES_PC], bf16)
                DOUT_BLK = 2   # dout chunks per weight block (256 cols)
                with (
                    tc.tile_pool(name="tokT", bufs=1) as tokT_pool,
                    tc.tile_pool(name="p1sb", bufs=4) as p1sb,
                    tc.tile_pool(name="wblk", bufs=2) as w_pool,
                    tc.tile_pool(name="mps", bufs=1, space="PSUM") as mps,
                ):
                    s0_t = p1sb.tile([128, KCH, HALF], bf16, tag="s0t", bufs=1)
                    nc.sync.dma_start(out=s0_t[:], in_=s0_d.ap())
                    iota_t = p1sb.tile([128, 1], f32, tag="iota_t", bufs=1)
                    nc.sync.dma_start(out=iota_t[:], in_=iota_t_d.ap())
                    def p1_graph(g, tokTh_hf, tokTl_hf):
                        gl = g % GH
                        lt = p1sb.tile([128, HALF], bf16, tag="lt", name="lt")
                        nc.vector.tensor_tensor(
                            out=lt[:], in0=blen_b[:, g * HALF:(g + 1) * HALF],
                            in1=iota_t[:].to_broadcast([128, HALF]), op=OP.is_gt)
                        ptok = mps.tile([64, BD], f32, tag="ptok", name="ptok", bufs=1)
                        for c in range(KCH):
                            bh_t = p1sb.tile([128, BD], bf16, tag="bh", name="bh")
                            bl_t = p1sb.tile([128, BD], bf16, tag="bl", name="bl")
                            r0 = g * (HALF * T) + c * 128
                            nc.sync.dma_start(out=bh_t[:], in_=bert_h_d.ap()[r0:r0 + 128, :])
                            nc.sync.dma_start(out=bl_t[:], in_=bert_l_d.ap()[r0:r0 + 128, :])
                            sm = p1sb.tile([128, HALF], bf16, tag="sm", name="sm")
                            nc.vector.tensor_tensor(out=sm[:], in0=s0_t[:, c, :], in1=lt[:],
                                                    op=OP.mult)
                            first, last = (c == 0), (c == KCH - 1)
                            nc.tensor.matmul(ptok[:, 0:512], lhsT=sm[:], rhs=bh_t[:, 0:512],
                                             start=first, stop=False)
                            nc.tensor.matmul(ptok[:, 0:512], lhsT=sm[:], rhs=bl_t[:, 0:512],
                                             start=False, stop=last)
                            nc.tensor.matmul(ptok[:, 512:768], lhsT=sm[:], rhs=bh_t[:, 512:768],
                                             start=first, stop=False)
                            nc.tensor.matmul(ptok[:, 512:768], lhsT=sm[:], rhs=bl_t[:, 512:768],
                                             start=False, stop=last)
                        tok_ng = p1sb.tile([64, BD], f32, tag="tokng", name="tokng", bufs=2)
                        nc.scalar.copy(out=tok_ng[:], in_=ptok[:])
                        for dcH in range(BC):
                            ptr = mps.tile([128, HALF], f32, tag="ptr", name="ptr", bufs=2)
                            nc.tensor.matmul(ptr[:], lhsT=tok_ng[:, dcH * 128:(dcH + 1) * 128],
                                             rhs=ident[0:64, 0:64], start=True, stop=True,
                                             is_transpose=True)
                            pair_write(tokTh_hf[:, dcH, gl * HALF:(gl + 1) * HALF],
                                       tokTl_hf[:, dcH, gl * HALF:(gl + 1) * HALF], ptr[:])

                    def dense_half(hf, tokTh_hf, tokTl_hf):
                        for dco in range(BC):
                            csl = slice(dco * 128, (dco + 1) * 128)
                            dwh = w_pool.tile([128, BC, 128], bf16, tag="wh", name="dwh")
                            dwl = w_pool.tile([128, BC, 128], bf16, tag="wl", name="dwl")
                            nc.sync.dma_start(
                                out=dwh[:],
                                in_=dWh_d.ap().rearrange("(c p) e -> p c e", p=128)[:, :, csl])
                            nc.sync.dma_start(
                                out=dwl[:],
                                in_=dWl_d.ap().rearrange("(c p) e -> p c e", p=128)[:, :, csl])
                            pd = mps.tile([128, GH * HALF], f32, tag="pqk", name="pd", bufs=3)
                            for dci in range(BC):
                                nc.tensor.matmul(pd[:], lhsT=dwh[:, dci, :],
                                                 rhs=tokTh_hf[:, dci, :],
                                                 start=(dci == 0), stop=False)
                                nc.tensor.matmul(pd[:], lhsT=dwh[:, dci, :],
                                                 rhs=tokTl_hf[:, dci, :], start=False, stop=False)
                                nc.tensor.matmul(pd[:], lhsT=dwl[:, dci, :],
                                                 rhs=tokTh_hf[:, dci, :], start=False,
                                                 stop=(dci == BC - 1))
                            gsl = slice(hf * GH, (hf + 1) * GH)
                            dsth = xTh4[:, dco, gsl, 0:HALF]
                            dstl = xTl4[:, dco, gsl, 0:HALF]
                            psrc = pd[:].rearrange("p (g h) -> p g h", h=HALF)
                            pair_write(dsth, dstl, psrc, bias=db32_t[:, dco:dco + 1])

                    def load_wblk(Wh_d_, Wl_d_, blk):
                        csl = slice(blk * DOUT_BLK * 128, (blk + 1) * DOUT_BLK * 128)
                        wh = w_pool.tile([128, DC, DOUT_BLK * 128], bf16, tag="wh", name="wh")
                        wl = w_pool.tile([128, DC, DOUT_BLK * 128], bf16, tag="wl", name="wl")
                        nc.sync.dma_start(
                            out=wh[:], in_=Wh_d_.ap().rearrange("(c p) e -> p c e", p=128)[:, :, csl])
                        nc.sync.dma_start(
                            out=wl[:], in_=Wl_d_.ap().rearrange("(c p) e -> p c e", p=128)[:, :, csl])
                        return wh, wl

                    def k_half(hf):
                        nsl = slice(hf * GH * L, (hf + 1) * GH * L)
                        for blk in range(DC // DOUT_BLK):
                            wh, wl = load_wblk(Wkh_d, Wkl_d, blk)
                            for j in range(DOUT_BLK):
                                dco = blk * DOUT_BLK + j
                                jsl = slice(j * 128, (j + 1) * 128)
                                pk = mps.tile([128, GH * L], f32, tag="pqk", name="pk", bufs=3)
                                for dci in range(DC):
                                    nc.tensor.matmul(pk[:], lhsT=wh[:, dci, jsl],
                                                     rhs=xTh[:, dci, nsl],
                                                     start=(dci == 0), stop=False)
                                    nc.tensor.matmul(pk[:], lhsT=wh[:, dci, jsl],
                                                     rhs=xTl[:, dci, nsl],
                                                     start=False, stop=False)
                                    nc.tensor.matmul(pk[:], lhsT=wl[:, dci, jsl],
                                                     rhs=xTh[:, dci, nsl],
                                                     start=False, stop=(dci == DC - 1))
                                pair_write(kTh[:, dco, nsl], kTl[:, dco, nsl], pk[:],
                                           bias=bk_t[:, dco:dco + 1])

                    def q_full():
                        for blk in range(DC // DOUT_BLK):
                            wh, wl = load_wblk(Wqh_d, Wql_d, blk)
                            for j in range(DOUT_BLK):
                                dco = blk * DOUT_BLK + j
                                jsl = slice(j * 128, (j + 1) * 128)
                                pq = mps.tile([128, G * HALF], f32, tag="pqk", name="pq", bufs=3)
                                for dci in range(DC):
                                    nc.tensor.matmul(pq[:], lhsT=wh[:, dci, jsl],
                                                     rhs=xTh4[:, dci, :, HALF:L],
                                                     start=(dci == 0), stop=False)
                                    nc.tensor.matmul(pq[:], lhsT=wh[:, dci, jsl],
                                                     rhs=xTl4[:, dci, :, HALF:L],
                                                     start=False, stop=False)
                                    nc.tensor.matmul(pq[:], lhsT=wl[:, dci, jsl],
                                                     rhs=xTh4[:, dci, :, HALF:L],
                                                     start=False, stop=(dci == DC - 1))
                                pair_write(qTh[:, dco, :], qTl[:, dco, :], pq[:],
                                           bias=bq_t[:, dco:dco + 1])

                    for hf in range(2):
                        tokTh_hf = tokT_pool.tile([128, BC, GH * HALF], bf16,
                                                  tag="tokTh", name="tokTh")
                        tokTl_hf = tokT_pool.tile([128, BC, GH * HALF], bf16,
                                                  tag="tokTl", name="tokTl")
                        for g in range(hf * GH, (hf + 1) * GH):
                            p1_graph(g, tokTh_hf, tokTl_hf)
                        dense_half(hf, tokTh_hf, tokTl_hf)
                        k_half(hf)
                    q_full()

                # =========================================================
                # P3: attention + top-3 per graph (clause query rows only)
                # =========================================================
                axT = adjT_pool.tile([128, DC, NAA], bf16)   # (A @ x)^T, clause rows
                with (
                    tc.tile_pool(name="att", bufs=2) as att_pool,
                    tc.tile_pool(name="attsc", bufs=2) as attsc_pool,
                    tc.tile_pool(name="p3ps", bufs=2, space="PSUM") as p3ps,
                ):
                    for g in range(G):
                        colmask = att_pool.tile([128, L], f32, tag="colmask")
                        nc.vector.tensor_scalar(
                            out=colmask[:], in0=col128[:], scalar1=glen_b[:, g:g + 1],
                            scalar2=-1e9, op0=OP.is_ge, op1=OP.mult)
                        vrow = attsc_pool.tile([128, 1], f32, tag="vrow")
                        nc.vector.tensor_scalar(
                            out=vrow[:], in0=iota_row[:], scalar1=glen_b[:, g:g + 1],
                            scalar2=1.0 / HEADS, op0=OP.is_lt, op1=OP.mult)
                        adj = att_pool.tile([64, L], f32, tag="adj")
                        for h in range(HEADS):
                            ps = p3ps.tile([64, L], f32, tag="pscore")
                            r0 = h * DK
                            chunks = [(ra, rb) for (ra, rb) in
                                      ((r0, min(r0 + DK, (r0 // 128 + 1) * 128)),
                                       ((r0 // 128 + 1) * 128, r0 + DK)) if rb > ra]
                            for ci, (ra, rb) in enumerate(chunks):
                                tI, p0 = ra // 128, ra % 128
                                p1_ = p0 + (rb - ra)
                                qs = slice(g * HALF, (g + 1) * HALF)
                                ks = slice(g * L, (g + 1) * L)
                                first = ci == 0
                                last = ci == len(chunks) - 1
                                nc.tensor.matmul(ps[:], lhsT=qTh[p0:p1_, tI, qs],
                                                 rhs=kTh[p0:p1_, tI, ks],
                                                 start=first, stop=False)
                                nc.tensor.matmul(ps[:], lhsT=qTh[p0:p1_, tI, qs],
                                                 rhs=kTl[p0:p1_, tI, ks],
                                                 start=False, stop=False)
                                nc.tensor.matmul(ps[:], lhsT=qTl[p0:p1_, tI, qs],
                                                 rhs=kTh[p0:p1_, tI, ks],
                                                 start=False, stop=last)
                            # mask invalid key columns; softmax over keys of s/sqrt(dk)
                            nc.vector.tensor_tensor(out=ps[:], in0=ps[:], in1=colmask[0:64, :],
                                                    op=OP.add)
                            negmax = attsc_pool.tile([64, 1], f32, tag="negmax")
                            nc.vector.reduce_max(out=negmax[:], in_=ps[:],
                                                 axis=mybir.AxisListType.X, negate=True)
                            nms = attsc_pool.tile([64, 1], f32, tag="nms")
                            nc.vector.tensor_scalar_mul(nms[:], negmax[:], INV_SQRT_DK)
                            exph = att_pool.tile([64, L], f32, tag="exph")
                            sumexp = attsc_pool.tile([64, 1], f32, tag="sumexp")
                            nc.scalar.activation(out=exph[:], in_=ps[:], func=AF.Exp,
                                                 bias=nms[:], scale=INV_SQRT_DK,
                                                 accum_out=sumexp[:])
                            recip = attsc_pool.tile([64, 1], f32, tag="recip")
                            nc.vector.reciprocal(out=recip[:], in_=sumexp[:])
                            if h == 0:
                                nc.vector.tensor_scalar(out=adj[:], in0=exph[:], scalar1=recip[:],
                                                        scalar2=None, op0=OP.mult)
                            else:
                                nc.vector.scalar_tensor_tensor(
                                    out=adj[:], in0=exph[:], scalar=recip[:], in1=adj[:],
                                    op0=OP.mult, op1=OP.add)
                        nc.vector.tensor_scalar(out=adj[:], in0=adj[:], scalar1=vrow[64:128, :],
                                                scalar2=None, op0=OP.mult)
                        # top-3 selection
                        top8 = attsc_pool.tile([64, 8], f32, tag="top8")
                        nc.vector.max(out=top8[:], in_=adj[:])
                        nc.vector.memset(top8[:, TOPK:8], 0.0)
                        zapped = att_pool.tile([64, L], f32, tag="zapped")
                        nc.vector.match_replace(out=zapped[:], in_to_replace=top8[:],
                                                in_values=adj[:], imm_value=0.0)
                        adjsel = att_pool.tile([64, L], f32, tag="adjsel")
                        denom = attsc_pool.tile([64, 1], f32, tag="denom")
                        nc.vector.tensor_tensor(out=adjsel[:], in0=adj[:], in1=zapped[:],
                                                op=OP.subtract)
                        nc.vector.reduce_sum(out=denom[:], in_=adjsel[:],
                                             axis=mybir.AxisListType.X)
                        nc.vector.tensor_scalar_add(denom[:], denom[:], 1.0)
                        recip_d = attsc_pool.tile([64, 1], f32, tag="recipd")
                        nc.vector.reciprocal(out=recip_d[:], in_=denom[:])
                        # adjT [128 nodes, 64 clause rows] in bf16
                        pat = p3ps.tile([128, 64], f32, tag="padjT")
                        nc.tensor.matmul(pat[:], lhsT=adjsel[:], rhs=ident[0:64, 0:64],
                                         start=True, stop=True, is_transpose=True)
                        nc.vector.tensor_copy(out=adjT_all[:, g, :], in_=pat[:])
                        # 1/denom -> row vector [1, 64]
                        prd = p3ps.tile([1, 64], f32, tag="pdenr", bufs=1)
                        nc.tensor.matmul(prd[:], lhsT=recip_d[:], rhs=ident[0:64, 0:64],
                                         start=True, stop=True, is_transpose=True)
                        nc.vector.tensor_copy(out=denr_row[:, g * HALF:(g + 1) * HALF], in_=prd[:])
                        # ---- Ax for this graph (plain bf16, post-selection)
                        xg = p3w.tile([128, D], bf16, tag="xg", name="xg")
                        for dcH in range(DC):
                            pxt = p3ps.tile([128, 128], bf16, tag="pxg", bufs=1)
                            nc.tensor.matmul(pxt[:], lhsT=xTh[:, dcH, g * L:(g + 1) * L],
                                             rhs=ident16[:], start=True, stop=True,
                                             is_transpose=True)
                            nc.scalar.copy(out=xg[:, dcH * 128:(dcH + 1) * 128], in_=pxt[:])
                        ax = p3w.tile([64, D], bf16, tag="ax", name="ax")
                        for nh in range(3):
                            pax = p3ps.tile([64, 512], f32, tag="pax", bufs=1)
                            nc.tensor.matmul(pax[:], lhsT=adjT_all[:, g, :],
                                             rhs=xg[:, nh * 512:(nh + 1) * 512],
                                             start=True, stop=True)
                            nc.scalar.copy(out=ax[:, nh * 512:(nh + 1) * 512], in_=pax[:])
                        for dcH in range(DC):
                            paxt = p3ps.tile([128, 64], bf16, tag="paxt", bufs=1)
                            nc.tensor.matmul(paxt[:], lhsT=ax[:, dcH * 128:(dcH + 1) * 128],
                                             rhs=ident16[0:64, 0:64], start=True, stop=True,
                                             is_transpose=True)
                            nc.scalar.copy(out=axT[:, dcH, g * HALF:(g + 1) * HALF], in_=paxt[:])

            # =========================================================
            # P4: GCN (bf16), epilogue, output
            # =========================================================
            with (
                tc.tile_pool(name="gw", bufs=2) as gw_pool,
                tc.tile_pool(name="ostage", bufs=1) as ost_pool,
                tc.tile_pool(name="gtmp", bufs=3) as gtmp_pool,
                tc.tile_pool(name="p4b", bufs=3, space="PSUM") as p4bps,
            ):
                denrb = gtmp_pool.tile([128, NAA], f32, tag="denrb", bufs=1)
                pb3 = p4bps.tile([128, NAA], f32, tag="pgcn")
                nc.tensor.matmul(pb3[:], lhsT=ones1[:], rhs=denr_row[:], start=True, stop=True)
                nc.vector.tensor_copy(out=denrb[:], in_=pb3[:])
                ostage = [ost_pool.tile([64, D], f32, tag=f"ost{g}", name=f"ost{g}")
                          for g in range(G)]
                GBLK = 3
                for blk in range(DC // GBLK):
                    csl = slice(blk * GBLK * 128, (blk + 1) * GBLK * 128)
                    gwt = gw_pool.tile([128, DC, GBLK * 128], bf16, tag="gw", name="gwt")
                    nc.sync.dma_start(
                        out=gwt[:], in_=gW_d.ap().rearrange("(c p) e -> p c e", p=128)[:, :, csl])
                    for j in range(GBLK):
                        dco = blk * GBLK + j
                        pg = p4bps.tile([128, NAA], f32, tag="pgcn")
                        for dci in range(DC):
                            nc.tensor.matmul(pg[:], lhsT=gwt[:, dci, j * 128:(j + 1) * 128],
                                             rhs=axT[:, dci, :],
                                             start=(dci == 0), stop=(dci == DC - 1))
                        trel = gtmp_pool.tile([128, NAA], f32, tag="trel")
                        nc.scalar.activation(out=trel[:], in_=pg[:], func=AF.Relu,
                                             bias=gb_t[:, dco:dco + 1], scale=1.0)
                        trel16 = gtmp_pool.tile([128, NAA], bf16, tag="trel16")
                        nc.vector.tensor_tensor(out=trel16[:], in0=trel[:], in1=denrb[:],
                                                op=OP.mult)
                        for g in range(G):
                            po = p4bps.tile([64, 128], bf16, tag="pout")
                            nc.tensor.matmul(po[:], lhsT=trel16[:, g * HALF:(g + 1) * HALF],
                                             rhs=ident16[:], start=True, stop=True,
                                             is_transpose=True)
                            nc.scalar.copy(out=ostage[g][:, dco * 128:(dco + 1) * 128],
                                           in_=po[:])
                for g in range(G):
                    nc.sync.dma_start(out=out_d.ap()[g * HALF:(g + 1) * HALF, :],
                                      in_=ostage[g][:])

    nc.compile()
    return nc


def _get_nc():
    if "nc" not in _STATE:
        _STATE["nc"] = _build_nc()
    return _STATE["nc"]


def _shard_inputs(inputs):
    """Split full inputs into 8 per-core maps (data-parallel over graphs),
    pre-splitting bf16 hi/lo pairs for BERT and the projection weights."""
    bert = np.ascontiguousarray(np.asarray(inputs["inner_bert_out"], dtype=np.float32))
    bert_h, bert_l = _split_pair(bert.reshape(B_TOTAL * HALF * T, BD))
    pooled = np.asarray(inputs["inner_pooled_out"], dtype=np.float32)
    clause = np.asarray(inputs["clause_output"], dtype=np.float32)
    blen = np.ascontiguousarray(np.asarray(inputs["batch_aa_bert_length"], dtype=np.int32))
    glen = np.ascontiguousarray(np.asarray(inputs["aa_graph_length"], dtype=np.int32))
    dWh, dWl = _split_pair(inputs["dense_W"])
    Wqh, Wql = _split_pair(inputs["Wq"])
    Wkh, Wkl = _split_pair(inputs["Wk"])
    gW16 = np.asarray(inputs["gcn_W"], dtype=np.float32).astype(BF16)
    reps = {
        "dense_W_h": dWh, "dense_W_l": dWl,
        "dense_b": np.asarray(inputs["dense_b"], np.float32),
        "Wq_h": Wqh, "Wq_l": Wql, "bq": np.asarray(inputs["bq"], np.float32),
        "Wk_h": Wkh, "Wk_l": Wkl, "bk": np.asarray(inputs["bk"], np.float32),
        "gcn_W16": gW16, "gcn_b": np.asarray(inputs["gcn_b"], np.float32),
    }
    in_maps = []
    rt = HALF * T
    for c in range(N_CORES):
        r0, r1 = c * NAA, (c + 1) * NAA
        pTh, pTl = _split_pair(np.ascontiguousarray(pooled[r0:r1].T))
        cTh, cTl = _split_pair(np.ascontiguousarray(clause[r0:r1].T))
        m = {
            "bert_h": bert_h[r0 * T:r1 * T],
            "bert_l": bert_l[r0 * T:r1 * T],
            "pooledT_h": pTh, "pooledT_l": pTl,
            "clauseT_h": cTh, "clauseT_l": cTl,
            "batch_aa_bert_length": blen[r0:r1],
            "aa_graph_length": glen[c * G:(c + 1) * G],
        }
        m.update(reps)
        in_maps.append(m)
    return in_maps


def kernel(**inputs) -> np.ndarray:
    from concourse.bass_utils import run_bass_kernel_spmd

    nc = _get_nc()
    in_maps = _shard_inputs(inputs)
    res = run_bass_kernel_spmd(nc, in_maps, core_ids=list(range(N_CORES)))
    return np.concatenate([res.results[c]["out"] for c in range(N_CORES)], axis=0)


# revision 46
# speedup vs baseline: 1.0811x; 1.0271x over previous
"""Trainium2 Bass kernel for nn_AA_encoder (gnn_message_passing).

Data-parallel over the B=64 graph dimension: 8 graphs per NeuronCore on a
single TRN2 chip (8 cores).  Per-core pipeline:

  P1  masked BERT token-sum (PE selector matmuls over host-pre-split
      bf16 hi/lo BERT pairs) -> tok_sum, transposed to tok_sumT pairs
  P1b dense projection (3-term bf16 split product, batched over graphs)
      + transposes of pooled/clause -> transposed node embeddings as
      bf16 hi/lo pairs xT_h/xT_l [1536, 8*128]
  P2  q/k projections as 3-term bf16 split products (weight pairs are
      host-pre-split); q only for the 64 clause query rows per graph
  P3  per-(graph, head) attention scores (3-term bf16 pair product) +
      masked softmax + head-mean, top-3 row selection via DVE
      max8/match_replace, adjacency transpose
  P4  Ax = adj_sel[clause rows] @ x and GCN matmul in plain bf16
      (post-selection values only need ~1e-3), bias/denominator/relu
      epilogue, transpose back to node-major rows, DMA out.

Precision scheme: everything upstream of the top-3 selection uses bf16
split pairs (v = bf16(v) + bf16(v - bf16(v)), ~17 mantissa bits) with
3-term products (h*h + l*h + h*l) accumulated in fp32 PSUM.  Measured on
the real generator distribution this reproduces the fp32 reference's
top-3 selections exactly (0/4096 flips); plain bf16 or tf32-style
rounding flips selections and fails the 2e-2 gate.  The TensorEngine
runs bf16 at 1 cycle/row vs fp32's 4, so this is ~2.7x less PE time.
"""

import numpy as np
import ml_dtypes

# ---------------- problem constants (hardcoded; kernel must be self-contained)
B_TOTAL = 64          # graphs
L = 128               # nodes per graph
HALF = 64
T = 32                # bert tokens per pair
BD = 768              # BERT_DIM == HIDDEN_DIM
D = 1536              # BD + HIDDEN
HEADS = 8
DK = D // HEADS       # 192
TOPK = 3
N_CORES = 8
G = B_TOTAL // N_CORES            # graphs per core = 8
NAA = G * HALF                    # AA/clause rows per core = 512
DC = D // 128                     # 12 chunks of the D dim
BC = BD // 128                    # 6 chunks of the BERT dim
KCH = (HALF * T) // 128           # 16 selector k-chunks per graph
NODES_PC = G * L                  # 1024 node columns per core
INV_SQRT_DK = 1.0 / float(np.sqrt(DK))

BF16 = ml_dtypes.bfloat16

_STATE = {}


def _split_pair(a):
    """v -> (bf16(v), bf16(v - bf16(v))) host-side split."""
    a = np.ascontiguousarray(np.asarray(a, dtype=np.float32))
    h = a.astype(BF16)
    l = (a - h.astype(np.float32)).astype(BF16)
    return h, l


def _build_nc():
    import concourse.bass as bass
    import concourse.bacc as bacc
    import concourse.mybir as mybir
    import concourse.tile as tile

    f32 = mybir.dt.float32
    bf16 = mybir.dt.bfloat16
    i32 = mybir.dt.int32
    AF = mybir.ActivationFunctionType
    OP = mybir.AluOpType

    nc = bacc.Bacc("TRN2", target_bir_lowering=False, debug=False)

    # ---------------- DRAM parameters (per-core shard shapes)
    bert_h_d = nc.dram_tensor("bert_h", [NAA * T, BD], bf16, kind="ExternalInput")
    bert_l_d = nc.dram_tensor("bert_l", [NAA * T, BD], bf16, kind="ExternalInput")
    pooledT_h_d = nc.dram_tensor("pooledT_h", [BD, NAA], bf16, kind="ExternalInput")
    pooledT_l_d = nc.dram_tensor("pooledT_l", [BD, NAA], bf16, kind="ExternalInput")
    clauseT_h_d = nc.dram_tensor("clauseT_h", [D, NAA], bf16, kind="ExternalInput")
    clauseT_l_d = nc.dram_tensor("clauseT_l", [D, NAA], bf16, kind="ExternalInput")
    blen_d = nc.dram_tensor("batch_aa_bert_length", [NAA], i32, kind="ExternalInput")
    glen_d = nc.dram_tensor("aa_graph_length", [G], i32, kind="ExternalInput")
    dWh_d = nc.dram_tensor("dense_W_h", [BD, BD], bf16, kind="ExternalInput")
    dWl_d = nc.dram_tensor("dense_W_l", [BD, BD], bf16, kind="ExternalInput")
    db_d = nc.dram_tensor("dense_b", [BD], f32, kind="ExternalInput")
    Wqh_d = nc.dram_tensor("Wq_h", [D, D], bf16, kind="ExternalInput")
    Wql_d = nc.dram_tensor("Wq_l", [D, D], bf16, kind="ExternalInput")
    bq_d = nc.dram_tensor("bq", [D], f32, kind="ExternalInput")
    Wkh_d = nc.dram_tensor("Wk_h", [D, D], bf16, kind="ExternalInput")
    Wkl_d = nc.dram_tensor("Wk_l", [D, D], bf16, kind="ExternalInput")
    bk_d = nc.dram_tensor("bk", [D], f32, kind="ExternalInput")
    gW_d = nc.dram_tensor("gcn_W16", [D, D], bf16, kind="ExternalInput")
    gb_d = nc.dram_tensor("gcn_b", [D], f32, kind="ExternalInput")
    out_d = nc.dram_tensor("out", [NAA, D], f32, kind="ExternalOutput")

    # ---------------- inline constants
    # S0[c, r, n] = 1 where n == 4c + r//32  (selector for 4 nodes per k-chunk)
    s0 = np.zeros((KCH, 128, HALF), np.float32)
    for c in range(KCH):
        for r in range(128):
            s0[c, r, 4 * c + r // 32] = 1.0
    s0_d = nc.inline_tensor(
        np.ascontiguousarray(s0.transpose(1, 0, 2)).astype(BF16), name="s0")  # [128,16,64]
    ident_d = nc.inline_tensor(np.eye(128, dtype=np.float32), name="ident")
    ident16_d = nc.inline_tensor(np.eye(128, dtype=np.float32).astype(BF16), name="ident16")
    iota_t_d = nc.inline_tensor(
        (np.arange(128, dtype=np.float32) % T).reshape(128, 1), name="iota_t")
    iota_row_d = nc.inline_tensor(
        np.arange(128, dtype=np.float32).reshape(128, 1), name="iota_row")
    col128_d = nc.inline_tensor(
        np.broadcast_to(np.arange(128, dtype=np.float32), (128, 128)).copy(), name="col128")
    ones1_d = nc.inline_tensor(np.ones((1, 128), np.float32), name="ones1")

    with tile.TileContext(nc) as tc:
        import contextlib
        with contextlib.ExitStack() as ctx:
            cpool = ctx.enter_context(tc.tile_pool(name="const", bufs=1))
            ident = cpool.tile([128, 128], f32)
            nc.sync.dma_start(out=ident[:], in_=ident_d.ap())
            ident16 = cpool.tile([128, 128], bf16)
            nc.sync.dma_start(out=ident16[:], in_=ident16_d.ap())
            iota_row = cpool.tile([128, 1], f32)
            nc.sync.dma_start(out=iota_row[:], in_=iota_row_d.ap())
            col128 = cpool.tile([128, 128], f32)
            nc.sync.dma_start(out=col128[:], in_=col128_d.ap())
            ones1 = cpool.tile([1, 128], f32)
            nc.sync.dma_start(out=ones1[:], in_=ones1_d.ap())

            # ---- biases as [128, chunks] column tiles
            db_t = cpool.tile([128, BC], f32)
            nc.sync.dma_start(out=db_t[:], in_=db_d.ap().rearrange("(c p) -> p c", p=128))
            db32_t = cpool.tile([128, BC], f32)
            nc.vector.tensor_scalar_mul(db32_t[:], db_t[:], float(T))
            bq_t = cpool.tile([128, DC], f32)
            nc.sync.dma_start(out=bq_t[:], in_=bq_d.ap().rearrange("(c p) -> p c", p=128))
            bk_t = cpool.tile([128, DC], f32)
            nc.sync.dma_start(out=bk_t[:], in_=bk_d.ap().rearrange("(c p) -> p c", p=128))
            gb_t = cpool.tile([128, DC], f32)
            nc.sync.dma_start(out=gb_t[:], in_=gb_d.ap().rearrange("(c p) -> p c", p=128))

            # ---- per-graph lengths broadcast across partitions
            glen_row = cpool.tile([1, G], f32)
            glen_i = cpool.tile([1, G], i32)
            nc.sync.dma_start(out=glen_i[:], in_=glen_d.ap().unsqueeze(0))
            nc.vector.tensor_copy(out=glen_row[:], in_=glen_i[:])
            blen_row = cpool.tile([1, NAA], f32)
            blen_i = cpool.tile([1, NAA], i32)
            nc.sync.dma_start(out=blen_i[:], in_=blen_d.ap().unsqueeze(0))
            nc.vector.tensor_copy(out=blen_row[:], in_=blen_i[:])

            with tc.tile_pool(name="bcast_ps", bufs=2, space="PSUM") as bps:
                glen_b = cpool.tile([128, G], f32)       # graph length on every partition
                pb = bps.tile([128, G], f32)
                nc.tensor.matmul(pb[:], lhsT=ones1[:], rhs=glen_row[:], start=True, stop=True)
                nc.vector.tensor_copy(out=glen_b[:], in_=pb[:])
                blen_b = cpool.tile([128, NAA], f32)     # per-pair bert length, bcast
                for h in range(NAA // 512):
                    pb2 = bps.tile([128, 512], f32)
                    nc.tensor.matmul(pb2[:], lhsT=ones1[:], rhs=blen_row[:, h * 512:(h + 1) * 512],
                                     start=True, stop=True)
                    nc.vector.tensor_copy(out=blen_b[:, h * 512:(h + 1) * 512], in_=pb2[:])

            # ---------------- persistent activation tiles (bf16 pairs)
            xT_pool = ctx.enter_context(tc.tile_pool(name="xT", bufs=1))
            xTh = xT_pool.tile([128, DC, NODES_PC], bf16)   # 3.1MB
            xTl = xT_pool.tile([128, DC, NODES_PC], bf16)   # 3.1MB

            def pair_write(dst_h, dst_l, psrc, bias=None):
                """dst_h = bf16(psrc + bias); dst_l = bf16((psrc + bias) - dst_h)."""
                if bias is None:
                    nc.scalar.copy(out=dst_h, in_=psrc)
                    nc.vector.tensor_tensor(out=dst_l, in0=psrc, in1=dst_h, op=OP.subtract)
                else:
                    nc.scalar.activation(out=dst_h, in_=psrc, func=AF.Identity,
                                         bias=bias, scale=1.0)
                    nc.vector.scalar_tensor_tensor(out=dst_l, in0=psrc, scalar=bias,
                                                   in1=dst_h, op0=OP.add, op1=OP.subtract)

            # ---- fill pooled/clause parts of xT straight from host-transposed
            #      bf16 pairs (no device transposes, no upstream deps)
            xTh5 = xTh[:].rearrange("p c (g l) -> p c g l", l=L)
            xTl5 = xTl[:].rearrange("p c (g l) -> p c g l", l=L)
            for dcH in range(BC):
                rs = slice(dcH * 128, (dcH + 1) * 128)
                nc.gpsimd.dma_start(
                    out=xTh5[:, BC + dcH, :, 0:HALF],
                    in_=pooledT_h_d.ap()[rs, :].rearrange("p (g n) -> p g n", n=HALF))
                nc.gpsimd.dma_start(
                    out=xTl5[:, BC + dcH, :, 0:HALF],
                    in_=pooledT_l_d.ap()[rs, :].rearrange("p (g n) -> p g n", n=HALF))
            for dcH in range(DC):
                rs = slice(dcH * 128, (dcH + 1) * 128)
                nc.gpsimd.dma_start(
                    out=xTh5[:, dcH, :, HALF:L],
                    in_=clauseT_h_d.ap()[rs, :].rearrange("p (g n) -> p g n", n=HALF))
                nc.gpsimd.dma_start(
                    out=xTl5[:, dcH, :, HALF:L],
                    in_=clauseT_l_d.ap()[rs, :].rearrange("p (g n) -> p g n", n=HALF))

            # =========================================================
            # P1+P2 interleaved by graph-halves: selector -> dense -> q/k
            # =========================================================
            xTh4 = xTh[:].rearrange("p c (g l) -> p c g l", l=L)
            xTl4 = xTl[:].rearrange("p c (g l) -> p c g l", l=L)
            adjT_pool = ctx.enter_context(tc.tile_pool(name="adjT", bufs=1))
            adjT_all = adjT_pool.tile([128, G, HALF], bf16)    # adj_sel^T per graph
            denr_row = cpool.tile([1, NAA], f32)               # 1/denom per clause row
            p3w = ctx.enter_context(tc.tile_pool(name="p3w", bufs=2))
            GH = G // 2                                        # graphs per half
            with tc.tile_pool(name="qkT", bufs=1) as qkT_pool:
                qTh = qkT_pool.tile([128, DC, G * HALF], bf16)
                qTl = qkT_pool.tile([128, DC, G * HALF], bf16)
                kTh = qkT_pool.tile([128, DC, NODES_PC], bf16)
                kTl = qkT_pool.tile([128, DC, NODES_PC], bf16)
                DOUT_BLK = 2   # dout chunks per weight block (256 cols)
                with (
                    tc.tile_pool(name="tokT", bufs=1) as tokT_pool,
                    tc.tile_pool(name="p1sb", bufs=4) as p1sb,
                    tc.tile_pool(name="wblk", bufs=2) as w_pool,
                    tc.tile_pool(name="mps", bufs=1, space="PSUM") as mps,
                ):
                    s0_t = p1sb.tile([128, KCH, HALF], bf16, tag="s0t", bufs=1)
                    nc.sync.dma_start(out=s0_t[:], in_=s0_d.ap())
                    iota_t = p1sb.tile([128, 1], f32, tag="iota_t", bufs=1)
                    nc.sync.dma_start(out=iota_t[:], in_=iota_t_d.ap())
                    def p1_graph(g, tokTh_hf, tokTl_hf):
                        gl = g % GH
                        lt = p1sb.tile([128, HALF], bf16, tag="lt", name="lt")
                        nc.vector.tensor_tensor(
                            out=lt[:], in0=blen_b[:, g * HALF:(g + 1) * HALF],
                            in1=iota_t[:].to_broadcast([128, HALF]), op=OP.is_gt)
                        ptok = mps.tile([64, BD], f32, tag="ptok", name="ptok", bufs=1)
                        for c in range(KCH):
                            bh_t = p1sb.tile([128, BD], bf16, tag="bh", name="bh")
                            bl_t = p1sb.tile([128, BD], bf16, tag="bl", name="bl")
                            r0 = g * (HALF * T) + c * 128
                            nc.sync.dma_start(out=bh_t[:], in_=bert_h_d.ap()[r0:r0 + 128, :])
                            nc.sync.dma_start(out=bl_t[:], in_=bert_l_d.ap()[r0:r0 + 128, :])
                            sm = p1sb.tile([128, HALF], bf16, tag="sm", name="sm")
                            nc.vector.tensor_tensor(out=sm[:], in0=s0_t[:, c, :], in1=lt[:],
                                                    op=OP.mult)
                            first, last = (c == 0), (c == KCH - 1)
                            nc.tensor.matmul(ptok[:, 0:512], lhsT=sm[:], rhs=bh_t[:, 0:512],
                                             start=first, stop=False)
                            nc.tensor.matmul(ptok[:, 0:512], lhsT=sm[:], rhs=bl_t[:, 0:512],
                                             start=False, stop=last)
                            nc.tensor.matmul(ptok[:, 512:768], lhsT=sm[:], rhs=bh_t[:, 512:768],
                                             start=first, stop=False)
                            nc.tensor.matmul(ptok[:, 512:768], lhsT=sm[:], rhs=bl_t[:, 512:768],
                                             start=False, stop=last)
                        tok_ng = p1sb.tile([64, BD], f32, tag="tokng", name="tokng", bufs=2)
                        nc.scalar.copy(out=tok_ng[:], in_=ptok[:])
                        for dcH in range(BC):
                            ptr = mps.tile([128, HALF], f32, tag="ptr", name="ptr", bufs=2)
                            nc.tensor.matmul(ptr[:], lhsT=tok_ng[:, dcH * 128:(dcH + 1) * 128],
                                             rhs=ident[0:64, 0:64], start=True, stop=True,
                                             is_transpose=True)
                            pair_write(tokTh_hf[:, dcH, gl * HALF:(gl + 1) * HALF],
                                       tokTl_hf[:, dcH, gl * HALF:(gl + 1) * HALF], ptr[:])

                    def dense_half(hf, tokTh_hf, tokTl_hf):
                        for dco in range(BC):
                            csl = slice(dco * 128, (dco + 1) * 128)
                            dwh = w_pool.tile([128, BC, 128], bf16, tag="wh", name="dwh")
                            dwl = w_pool.tile([128, BC, 128], bf16, tag="wl", name="dwl")
                            nc.gpsimd.dma_start(
                                out=dwh[:],
                                in_=dWh_d.ap().rearrange("(c p) e -> p c e", p=128)[:, :, csl])
                            nc.gpsimd.dma_start(
                                out=dwl[:],
                                in_=dWl_d.ap().rearrange("(c p) e -> p c e", p=128)[:, :, csl])
                            pd = mps.tile([128, GH * HALF], f32, tag="pqk", name="pd", bufs=3)
                            for dci in range(BC):
                                nc.tensor.matmul(pd[:], lhsT=dwh[:, dci, :],
                                                 rhs=tokTh_hf[:, dci, :],
                                                 start=(dci == 0), stop=False)
                                nc.tensor.matmul(pd[:], lhsT=dwh[:, dci, :],
                                                 rhs=tokTl_hf[:, dci, :], start=False, stop=False)
                                nc.tensor.matmul(pd[:], lhsT=dwl[:, dci, :],
                                                 rhs=tokTh_hf[:, dci, :], start=False,
                                                 stop=(dci == BC - 1))
                            gsl = slice(hf * GH, (hf + 1) * GH)
                            dsth = xTh4[:, dco, gsl, 0:HALF]
                            dstl = xTl4[:, dco, gsl, 0:HALF]
                            psrc = pd[:].rearrange("p (g h) -> p g h", h=HALF)
                            pair_write(dsth, dstl, psrc, bias=db32_t[:, dco:dco + 1])

                    def load_wblk(Wh_d_, Wl_d_, blk):
                        csl = slice(blk * DOUT_BLK * 128, (blk + 1) * DOUT_BLK * 128)
                        wh = w_pool.tile([128, DC, DOUT_BLK * 128], bf16, tag="wh", name="wh")
                        wl = w_pool.tile([128, DC, DOUT_BLK * 128], bf16, tag="wl", name="wl")
                        nc.gpsimd.dma_start(
                            out=wh[:], in_=Wh_d_.ap().rearrange("(c p) e -> p c e", p=128)[:, :, csl])
                        nc.gpsimd.dma_start(
                            out=wl[:], in_=Wl_d_.ap().rearrange("(c p) e -> p c e", p=128)[:, :, csl])
                        return wh, wl

                    def k_half(hf):
                        nsl = slice(hf * GH * L, (hf + 1) * GH * L)
                        for blk in range(DC // DOUT_BLK):
                            wh, wl = load_wblk(Wkh_d, Wkl_d, blk)
                            for j in range(DOUT_BLK):
                                dco = blk * DOUT_BLK + j
                                jsl = slice(j * 128, (j + 1) * 128)
                                pk = mps.tile([128, GH * L], f32, tag="pqk", name="pk", bufs=3)
                                for dci in range(DC):
                                    nc.tensor.matmul(pk[:], lhsT=wh[:, dci, jsl],
                                                     rhs=xTh[:, dci, nsl],
                                                     start=(dci == 0), stop=False)
                                    nc.tensor.matmul(pk[:], lhsT=wh[:, dci, jsl],
                                                     rhs=xTl[:, dci, nsl],
                                                     start=False, stop=False)
                                    nc.tensor.matmul(pk[:], lhsT=wl[:, dci, jsl],
                                                     rhs=xTh[:, dci, nsl],
                                                     start=False, stop=(dci == DC - 1))
                                pair_write(kTh[:, dco, nsl], kTl[:, dco, nsl], pk[:],
                                           bias=bk_t[:, dco:dco + 1])

                    def q_full():
                        for blk in range(DC // DOUT_BLK):
                            wh, wl = load_wblk(Wqh_d, Wql_d, blk)
                            for j in range(DOUT_BLK):
                                dco = blk * DOUT_BLK + j
                                jsl = slice(j * 128, (j + 1) * 128)
                                pq = mps.tile([128, G * HALF], f32, tag="pqk", name="pq", bufs=3)
                                for dci in range(DC):
                                    nc.tensor.matmul(pq[:], lhsT=wh[:, dci, jsl],
                                                     rhs=xTh4[:, dci, :, HALF:L],
                                                     start=(dci == 0), stop=False)
                                    nc.tensor.matmul(pq[:], lhsT=wh[:, dci, jsl],
                                                     rhs=xTl4[:, dci, :, HALF:L],
                                                     start=False, stop=False)
                                    nc.tensor.matmul(pq[:], lhsT=wl[:, dci, jsl],
                                                     rhs=xTh4[:, dci, :, HALF:L],
                                                     start=False, stop=(dci == DC - 1))
                                pair_write(qTh[:, dco, :], qTl[:, dco, :], pq[:],
                                           bias=bq_t[:, dco:dco + 1])

                    for hf in range(2):
                        tokTh_hf = tokT_pool.tile([128, BC, GH * HALF], bf16,
                                                  tag="tokTh", name="tokTh")
                        tokTl_hf = tokT_pool.tile([128, BC, GH * HALF], bf16,
                                                  tag="tokTl", name="tokTl")
                        for g in range(hf * GH, (hf + 1) * GH):
                            p1_graph(g, tokTh_hf, tokTl_hf)
                        dense_half(hf, tokTh_hf, tokTl_hf)
                        k_half(hf)
                    q_full()

                # =========================================================
                # P3: attention + top-3 per graph (clause query rows only)
                # =========================================================
                axT = adjT_pool.tile([128, DC, NAA], bf16)   # (A @ x)^T, clause rows
                with (
                    tc.tile_pool(name="att", bufs=2) as att_pool,
                    tc.tile_pool(name="attsc", bufs=2) as attsc_pool,
                    tc.tile_pool(name="p3ps", bufs=2, space="PSUM") as p3ps,
                ):
                    for g in range(G):
                        colmask = att_pool.tile([128, L], f32, tag="colmask")
                        nc.vector.tensor_scalar(
                            out=colmask[:], in0=col128[:], scalar1=glen_b[:, g:g + 1],
                            scalar2=-1e9, op0=OP.is_ge, op1=OP.mult)
                        vrow = attsc_pool.tile([128, 1], f32, tag="vrow")
                        nc.vector.tensor_scalar(
                            out=vrow[:], in0=iota_row[:], scalar1=glen_b[:, g:g + 1],
                            scalar2=1.0 / HEADS, op0=OP.is_lt, op1=OP.mult)
                        adj = att_pool.tile([64, L], f32, tag="adj")
                        for h in range(HEADS):
                            ps = p3ps.tile([64, L], f32, tag="pscore")
                            r0 = h * DK
                            chunks = [(ra, rb) for (ra, rb) in
                                      ((r0, min(r0 + DK, (r0 // 128 + 1) * 128)),
                                       ((r0 // 128 + 1) * 128, r0 + DK)) if rb > ra]
                            for ci, (ra, rb) in enumerate(chunks):
                                tI, p0 = ra // 128, ra % 128
                                p1_ = p0 + (rb - ra)
                                qs = slice(g * HALF, (g + 1) * HALF)
                                ks = slice(g * L, (g + 1) * L)
                                first = ci == 0
                                last = ci == len(chunks) - 1
                                nc.tensor.matmul(ps[:], lhsT=qTh[p0:p1_, tI, qs],
                                                 rhs=kTh[p0:p1_, tI, ks],
                                                 start=first, stop=False)
                                nc.tensor.matmul(ps[:], lhsT=qTh[p0:p1_, tI, qs],
                                                 rhs=kTl[p0:p1_, tI, ks],
                                                 start=False, stop=False)
                                nc.tensor.matmul(ps[:], lhsT=qTl[p0:p1_, tI, qs],
                                                 rhs=kTh[p0:p1_, tI, ks],
                                                 start=False, stop=last)
                            # mask invalid key columns; softmax over keys of s/sqrt(dk)
                            nc.vector.tensor_tensor(out=ps[:], in0=ps[:], in1=colmask[0:64, :],
                                                    op=OP.add)
                            negmax = attsc_pool.tile([64, 1], f32, tag="negmax")
                            nc.vector.reduce_max(out=negmax[:], in_=ps[:],
                                                 axis=mybir.AxisListType.X, negate=True)
                            nms = attsc_pool.tile([64, 1], f32, tag="nms")
                            nc.vector.tensor_scalar_mul(nms[:], negmax[:], INV_SQRT_DK)
                            exph = att_pool.tile([64, L], f32, tag="exph")
                            sumexp = attsc_pool.tile([64, 1], f32, tag="sumexp")
                            nc.scalar.activation(out=exph[:], in_=ps[:], func=AF.Exp,
                                                 bias=nms[:], scale=INV_SQRT_DK,
                                                 accum_out=sumexp[:])
                            recip = attsc_pool.tile([64, 1], f32, tag="recip")
                            nc.vector.reciprocal(out=recip[:], in_=sumexp[:])
                            if h == 0:
                                nc.vector.tensor_scalar(out=adj[:], in0=exph[:], scalar1=recip[:],
                                                        scalar2=None, op0=OP.mult)
                            else:
                                nc.vector.scalar_tensor_tensor(
                                    out=adj[:], in0=exph[:], scalar=recip[:], in1=adj[:],
                                    op0=OP.mult, op1=OP.add)
                        nc.vector.tensor_scalar(out=adj[:], in0=adj[:], scalar1=vrow[64:128, :],
                                                scalar2=None, op0=OP.mult)
                        # top-3 selection
                        top8 = attsc_pool.tile([64, 8], f32, tag="top8")
                        nc.vector.max(out=top8[:], in_=adj[:])
                        nc.vector.memset(top8[:, TOPK:8], 0.0)
                        zapped = att_pool.tile([64, L], f32, tag="zapped")
                        nc.vector.match_replace(out=zapped[:], in_to_replace=top8[:],
                                                in_values=adj[:], imm_value=0.0)
                        adjsel = att_pool.tile([64, L], f32, tag="adjsel")
                        denom = attsc_pool.tile([64, 1], f32, tag="denom")
                        nc.vector.tensor_tensor(out=adjsel[:], in0=adj[:], in1=zapped[:],
                                                op=OP.subtract)
                        nc.vector.reduce_sum(out=denom[:], in_=adjsel[:],
                                             axis=mybir.AxisListType.X)
                        nc.vector.tensor_scalar_add(denom[:], denom[:], 1.0)
                        recip_d = attsc_pool.tile([64, 1], f32, tag="recipd")
                        nc.vector.reciprocal(out=recip_d[:], in_=denom[:])
                        # adjT [128 nodes, 64 clause rows] in bf16
                        pat = p3ps.tile([128, 64], f32, tag="padjT")
                        nc.tensor.matmul(pat[:], lhsT=adjsel[:], rhs=ident[0:64, 0:64],
                                         start=True, stop=True, is_transpose=True)
                        nc.vector.tensor_copy(out=adjT_all[:, g, :], in_=pat[:])
                        # 1/denom -> row vector [1, 64]
                        prd = p3ps.tile([1, 64], f32, tag="pdenr", bufs=1)
                        nc.tensor.matmul(prd[:], lhsT=recip_d[:], rhs=ident[0:64, 0:64],
                                         start=True, stop=True, is_transpose=True)
                        nc.vector.tensor_copy(out=denr_row[:, g * HALF:(g + 1) * HALF], in_=prd[:])
                        # ---- Ax for this graph (plain bf16, post-selection)
                        xg = p3w.tile([128, D], bf16, tag="xg", name="xg")
                        for dcH in range(DC):
                            pxt = p3ps.tile([128, 128], bf16, tag="pxg", bufs=1)
                            nc.tensor.matmul(pxt[:], lhsT=xTh[:, dcH, g * L:(g + 1) * L],
                                             rhs=ident16[:], start=True, stop=True,
                                             is_transpose=True)
                            nc.scalar.copy(out=xg[:, dcH * 128:(dcH + 1) * 128], in_=pxt[:])
                        ax = p3w.tile([64, D], bf16, tag="ax", name="ax")
                        for nh in range(3):
                            pax = p3ps.tile([64, 512], f32, tag="pax", bufs=1)
                            nc.tensor.matmul(pax[:], lhsT=adjT_all[:, g, :],
                                             rhs=xg[:, nh * 512:(nh + 1) * 512],
                                             start=True, stop=True)
                            nc.scalar.copy(out=ax[:, nh * 512:(nh + 1) * 512], in_=pax[:])
                        for dcH in range(DC):
                            paxt = p3ps.tile([128, 64], bf16, tag="paxt", bufs=1)
                            nc.tensor.matmul(paxt[:], lhsT=ax[:, dcH * 128:(dcH + 1) * 128],
                                             rhs=ident16[0:64, 0:64], start=True, stop=True,
                                             is_transpose=True)
                            nc.scalar.copy(out=axT[:, dcH, g * HALF:(g + 1) * HALF], in_=paxt[:])

            # =========================================================
            # P4: GCN (bf16), epilogue, output
            # =========================================================
            with (
                tc.tile_pool(name="gw", bufs=2) as gw_pool,
                tc.tile_pool(name="ostage", bufs=1) as ost_pool,
                tc.tile_pool(name="gtmp", bufs=3) as gtmp_pool,
                tc.tile_pool(name="p4b", bufs=3, space="PSUM") as p4bps,
            ):
                denrb = gtmp_pool.tile([128, NAA], f32, tag="denrb", bufs=1)
                pb3 = p4bps.tile([128, NAA], f32, tag="pgcn")
                nc.tensor.matmul(pb3[:], lhsT=ones1[:], rhs=denr_row[:], start=True, stop=True)
                nc.vector.tensor_copy(out=denrb[:], in_=pb3[:])
                ostage = [ost_pool.tile([64, D], f32, tag=f"ost{g}", name=f"ost{g}")
                          for g in range(G)]
                GBLK = 3
                for blk in range(DC // GBLK):
                    csl = slice(blk * GBLK * 128, (blk + 1) * GBLK * 128)
                    gwt = gw_pool.tile([128, DC, GBLK * 128], bf16, tag="gw", name="gwt")
                    nc.gpsimd.dma_start(
                        out=gwt[:], in_=gW_d.ap().rearrange("(c p) e -> p c e", p=128)[:, :, csl])
                    for j in range(GBLK):
                        dco = blk * GBLK + j
                        pg = p4bps.tile([128, NAA], f32, tag="pgcn")
                        for dci in range(DC):
                            nc.tensor.matmul(pg[:], lhsT=gwt[:, dci, j * 128:(j + 1) * 128],
                                             rhs=axT[:, dci, :],
                                             start=(dci == 0), stop=(dci == DC - 1))
                        trel = gtmp_pool.tile([128, NAA], f32, tag="trel")
                        nc.scalar.activation(out=trel[:], in_=pg[:], func=AF.Relu,
                                             bias=gb_t[:, dco:dco + 1], scale=1.0)
                        trel16 = gtmp_pool.tile([128, NAA], bf16, tag="trel16")
                        nc.vector.tensor_tensor(out=trel16[:], in0=trel[:], in1=denrb[:],
                                                op=OP.mult)
                        for g in range(G):
                            po = p4bps.tile([64, 128], bf16, tag="pout")
                            nc.tensor.matmul(po[:], lhsT=trel16[:, g * HALF:(g + 1) * HALF],
                                             rhs=ident16[:], start=True, stop=True,
                                             is_transpose=True)
                            nc.scalar.copy(out=ostage[g][:, dco * 128:(dco + 1) * 128],
                                           in_=po[:])
                for g in range(G):
                    nc.sync.dma_start(out=out_d.ap()[g * HALF:(g + 1) * HALF, :],
                                      in_=ostage[g][:])

    nc.compile()
    return nc


def _get_nc():
    if "nc" not in _STATE:
        _STATE["nc"] = _build_nc()
    return _STATE["nc"]


def _shard_inputs(inputs):
    """Split full inputs into 8 per-core maps (data-parallel over graphs),
    pre-splitting bf16 hi/lo pairs for BERT and the projection weights."""
    bert = np.ascontiguousarray(np.asarray(inputs["inner_bert_out"], dtype=np.float32))
    bert_h, bert_l = _split_pair(bert.reshape(B_TOTAL * HALF * T, BD))
    pooled = np.asarray(inputs["inner_pooled_out"], dtype=np.float32)
    clause = np.asarray(inputs["clause_output"], dtype=np.float32)
    blen = np.ascontiguousarray(np.asarray(inputs["batch_aa_bert_length"], dtype=np.int32))
    glen = np.ascontiguousarray(np.asarray(inputs["aa_graph_length"], dtype=np.int32))
    dWh, dWl = _split_pair(inputs["dense_W"])
    Wqh, Wql = _split_pair(inputs["Wq"])
    Wkh, Wkl = _split_pair(inputs["Wk"])
    gW16 = np.asarray(inputs["gcn_W"], dtype=np.float32).astype(BF16)
    reps = {
        "dense_W_h": dWh, "dense_W_l": dWl,
        "dense_b": np.asarray(inputs["dense_b"], np.float32),
        "Wq_h": Wqh, "Wq_l": Wql, "bq": np.asarray(inputs["bq"], np.float32),
        "Wk_h": Wkh, "Wk_l": Wkl, "bk": np.asarray(inputs["bk"], np.float32),
        "gcn_W16": gW16, "gcn_b": np.asarray(inputs["gcn_b"], np.float32),
    }
    in_maps = []
    rt = HALF * T
    for c in range(N_CORES):
        r0, r1 = c * NAA, (c + 1) * NAA
        pTh, pTl = _split_pair(np.ascontiguousarray(pooled[r0:r1].T))
        cTh, cTl = _split_pair(np.ascontiguousarray(clause[r0:r1].T))
        m = {
            "bert_h": bert_h[r0 * T:r1 * T],
            "bert_l": bert_l[r0 * T:r1 * T],
            "pooledT_h": pTh, "pooledT_l": pTl,
            "clauseT_h": cTh, "clauseT_l": cTl,
            "batch_aa_bert_length": blen[r0:r1],
            "aa_graph_length": glen[c * G:(c + 1) * G],
        }
        m.update(reps)
        in_maps.append(m)
    return in_maps


def kernel(**inputs) -> np.ndarray:
    from concourse.bass_utils import run_bass_kernel_spmd

    nc = _get_nc()
    in_maps = _shard_inputs(inputs)
    res = run_bass_kernel_spmd(nc, in_maps, core_ids=list(range(N_CORES)))
    return np.concatenate([res.results[c]["out"] for c in range(N_CORES)], axis=0)
